# revision 3
# baseline (speedup 1.0000x reference)
"""Trainium2 Bass kernel for the AnaphoricityScorer (coref pairwise FFNN scorer).

Math (per batch row i, antecedent slot t):
    b  = all_mentions[top_indices[i, t]]                    # gathered mention
    pair = [a_i, b, a_i * b, pw[i, t]]                      # 3*1024 + 64 features
    h  = leaky_relu(pair @ W1.T + b1, 0.01)                 # 1024 hidden
    ffnn = h @ Wout.T + bout                                # scalar
    score = rough[i, t] + ffnn
    out = concat([eps_col, scores], axis=1)                 # [batch, 65]

Distribution: pure data parallel over the batch dim across 8 NeuronCores
(no collectives). all_mentions and FFNN weights are replicated.

Per-core algorithm (B = 128 batch rows -> 8192 pair rows, groups of 512):
  - The a-term a_i @ W1a.T is identical for all 64 antecedents of row i, so it
    is computed once per batch row in a prologue (ha = mentions @ W1a.T + b1)
    and broadcast-added into the pair-row PSUM with a stride-0 DVE add.
  - b rows arrive transposed (features on partitions) straight from HBM via
    dma_gather(transpose=True), which is exactly the matmul rhs layout.
  - a*b is built by a DVE multiply against a stride-0 broadcast of mentions^T.
  - One PSUM accumulation of 17 matmuls per (row-group, hidden-tile):
    8 K-tiles of W1b, 8 of W1ab, 1 of W1pw (K=64).
  - Lrelu on ScalarE evicts PSUM -> SBUF bf16; the Wout reduction is a
    K=128, M=1 matmul accumulated over the 8 hidden tiles.
  - bout + rough scores are added on DVE; one DMA out per core.

Everything is bf16 on the TensorEngine with fp32 PSUM accumulation.
"""

import sys

for _p in ("/opt/trn_rl_repo",):
    if _p not in sys.path:
        sys.path.append(_p)

import numpy as np
import ml_dtypes

import concourse.bacc as bacc
import concourse.mybir as mybir
from concourse.tile import TileContext
from concourse.bass_utils import run_bass_kernel_spmd

BF16 = mybir.dt.bfloat16
F32 = mybir.dt.float32
I16 = mybir.dt.int16

N_CORES = 8
EMB = 1024
HID = 1024
N_ANTS = 64
PW = 64
EPS = 1e-7
GRP = 512          # pair rows per group (= 8 batch rows)
ROWS_PER_GRP = 8   # batch rows per group


def build_nc(B: int, n_tab: int):
    """Build the per-core Bass graph. B = batch rows per core."""
    G = (B * N_ANTS) // GRP  # number of row groups
    FC = EMB // 128          # 8 feature k-tiles per 1024-feature block
    NT = HID // 128          # 8 hidden tiles

    nc = bacc.Bacc("TRN2")
    amen = nc.declare_dram_parameter("amen", [n_tab, EMB], BF16, isOutput=False)
    ment = nc.declare_dram_parameter("ment", [128, FC, B], BF16, isOutput=False)
    w1bt = nc.declare_dram_parameter("w1bt", [128, FC, HID], BF16, isOutput=False)
    w1abt = nc.declare_dram_parameter("w1abt", [128, FC, HID], BF16, isOutput=False)
    w1at = nc.declare_dram_parameter("w1at", [128, FC, HID], BF16, isOutput=False)
    w1pw = nc.declare_dram_parameter("w1pw", [PW, HID], BF16, isOutput=False)
    b1t = nc.declare_dram_parameter("b1t", [128, NT], F32, isOutput=False)
    woutt = nc.declare_dram_parameter("woutt", [128, NT], BF16, isOutput=False)
    pwt = nc.declare_dram_parameter("pwt", [PW, B * N_ANTS], BF16, isOutput=False)
    idx = nc.declare_dram_parameter("idx", [128, G * (GRP // 16)], I16, isOutput=False)
    rough = nc.declare_dram_parameter("rough", [1, B * N_ANTS], F32, isOutput=False)
    out = nc.declare_dram_parameter("out", [B, N_ANTS], F32, isOutput=True)

    with TileContext(nc) as tc:
        with (
            tc.tile_pool(name="const", bufs=1) as const,
            tc.tile_pool(name="btp", bufs=3) as btp,
            tc.tile_pool(name="abtp", bufs=3) as abtp,
            tc.tile_pool(name="htp", bufs=4) as htp,
            tc.tile_pool(name="psum", bufs=3, space="PSUM") as psum_pool,
            tc.tile_pool(name="psum_s", bufs=2, space="PSUM") as psum_s_pool,
        ):
            # ---- resident loads -------------------------------------------
            ment_t = const.tile([128, FC, B], BF16)
            nc.sync.dma_start(ment_t[:], ment[:, :, :])
            w1bt_t = const.tile([128, FC, HID], BF16)
            nc.sync.dma_start(w1bt_t[:], w1bt[:, :, :])
            w1abt_t = const.tile([128, FC, HID], BF16)
            nc.sync.dma_start(w1abt_t[:], w1abt[:, :, :])
            w1at_t = const.tile([128, FC, HID], BF16)
            nc.sync.dma_start(w1at_t[:], w1at[:, :, :])
            w1pw_t = const.tile([PW, HID], BF16)
            nc.sync.dma_start(w1pw_t[:], w1pw[:, :])
            b1t_t = const.tile([128, NT], F32)
            nc.sync.dma_start(b1t_t[:], b1t[:, :])
            woutt_t = const.tile([128, NT], BF16)
            nc.sync.dma_start(woutt_t[:], woutt[:, :])
            pwt_t = const.tile([PW, B * N_ANTS], BF16)
            nc.sync.dma_start(pwt_t[:], pwt[:, :])
            idx_t = const.tile([128, G * (GRP // 16)], I16)
            nc.sync.dma_start(idx_t[:], idx[:, :])
            rough_t = const.tile([1, B * N_ANTS], F32)
            nc.sync.dma_start(rough_t[:], rough[:, :])

            scores_t = const.tile([1, B * N_ANTS], F32)
            haT = const.tile([128, NT, B], F32)  # ha.T + b1, [hid%128, hidtile, row]

            # ---- prologue: ha.T = mentions @ W1a.T + b1 -------------------
            for nt in range(NT):
                pp = psum_pool.tile([128, B], F32)
                for fc in range(FC):
                    nc.tensor.matmul(
                        pp[:],
                        w1at_t[:, fc, nt * 128:(nt + 1) * 128],
                        ment_t[:, fc, :],
                        start=(fc == 0),
                        stop=(fc == FC - 1),
                    )
                nc.scalar.activation(
                    haT[:, nt, :], pp[:],
                    mybir.ActivationFunctionType.Identity,
                    bias=b1t_t[:, nt:nt + 1],
                )

            # ---- main loop over row groups --------------------------------
            for g in range(G):
                r0 = g * ROWS_PER_GRP
                bt = btp.tile([128, FC, GRP], BF16)
                nc.gpsimd.dma_gather(
                    bt[:], amen[:, :],
                    idx_t[:, g * (GRP // 16):(g + 1) * (GRP // 16)],
                    GRP, GRP, EMB, transpose=True,
                )
                abt = abtp.tile([128, FC, GRP], BF16)
                a_b = ment_t[:, :, r0:r0 + ROWS_PER_GRP]
                for fc in range(FC):
                    nc.vector.tensor_mul(
                        abt[:, fc, :].rearrange("p (a b) -> p a b", a=ROWS_PER_GRP),
                        bt[:, fc, :].rearrange("p (a b) -> p a b", a=ROWS_PER_GRP),
                        a_b[:, fc, :].unsqueeze(2).to_broadcast(
                            [128, ROWS_PER_GRP, N_ANTS]),
                    )

                ps_s = psum_s_pool.tile([1, GRP], F32)
                for nt in range(NT):
                    ps = psum_pool.tile([128, GRP], F32)
                    nsl = slice(nt * 128, (nt + 1) * 128)
                    for fc in range(FC):
                        nc.tensor.matmul(
                            ps[:], w1bt_t[:, fc, nsl], bt[:, fc, :],
                            start=(fc == 0), stop=False,
                        )
                    for fc in range(FC):
                        nc.tensor.matmul(
                            ps[:], w1abt_t[:, fc, nsl], abt[:, fc, :],
                            start=False, stop=False,
                        )
                    nc.tensor.matmul(
                        ps[:], w1pw_t[:, nsl],
                        pwt_t[:, g * GRP:(g + 1) * GRP],
                        start=False, stop=True,
                    )
                    # broadcast-add the per-batch-row a-term (incl. b1)
                    nc.vector.tensor_add(
                        ps[:].rearrange("p (a b) -> p a b", a=ROWS_PER_GRP),
                        ps[:].rearrange("p (a b) -> p a b", a=ROWS_PER_GRP),
                        haT[:, nt, r0:r0 + ROWS_PER_GRP].unsqueeze(2).to_broadcast(
                            [128, ROWS_PER_GRP, N_ANTS]),
                    )
                    ht = htp.tile([128, GRP], BF16)
                    nc.scalar.activation(
                        ht[:], ps[:],
                        mybir.ActivationFunctionType.Lrelu, alpha=0.01,
                    )
                    nc.tensor.matmul(
                        ps_s[:], woutt_t[:, nt:nt + 1], ht[:],
                        start=(nt == 0), stop=(nt == NT - 1),
                    )
                nc.vector.tensor_add(
                    scores_t[0:1, g * GRP:(g + 1) * GRP],
                    ps_s[0:1, :],
                    rough_t[0:1, g * GRP:(g + 1) * GRP],
                )

            nc.sync.dma_start(
                out[:, :].unsqueeze(0),
                scores_t[0:1, :].rearrange("p (a b) -> p a b", a=B),
            )

    nc.compile()
    return nc


def prep_inputs(all_mentions, mentions_batch, pw_batch, top_indices_batch,
                top_rough_scores_batch, W1, b1, Wout, bout, n_cores=N_CORES):
    """Host-side marshalling: shard over batch, cast/transpose into the
    layouts the kernel expects. Returns (in_maps, B, n_tab, bout_val)."""
    bf = ml_dtypes.bfloat16
    batch = mentions_batch.shape[0]
    B = batch // n_cores
    n_tab = all_mentions.shape[0]
    FC = EMB // 128
    NT = HID // 128
    G = (B * N_ANTS) // GRP

    amen = np.ascontiguousarray(all_mentions.astype(bf))

    def wt_block(Wcols):  # [1024, 1024] f32 block -> [128, FC, HID] bf16 (f on part)
        wt = Wcols.T.reshape(FC, 128, HID).transpose(1, 0, 2)
        return np.ascontiguousarray(wt.astype(bf))

    w1at = wt_block(W1[:, 0:EMB])
    w1bt = wt_block(W1[:, EMB:2 * EMB])
    w1abt = wt_block(W1[:, 2 * EMB:3 * EMB])
    w1pw = np.ascontiguousarray(W1[:, 3 * EMB:3 * EMB + PW].T.astype(bf))
    b1t = np.ascontiguousarray(b1.reshape(NT, 128).T.astype(np.float32))
    woutt = np.ascontiguousarray(Wout[0].reshape(NT, 128).T.astype(bf))

    in_maps = []
    for c in range(n_cores):
        rows = slice(c * B, (c + 1) * B)
        m_c = np.asarray(mentions_batch[rows], dtype=np.float32)       # [B, 1024]
        ment = np.ascontiguousarray(
            m_c.T.reshape(FC, 128, B).transpose(1, 0, 2).astype(bf))   # [128, FC, B]
        pw_c = np.asarray(pw_batch[rows], dtype=np.float32)            # [B, 64, 64]
        pwt = np.ascontiguousarray(pw_c.reshape(B * N_ANTS, PW).T.astype(bf))
        idx_c = np.asarray(top_indices_batch[rows]).astype(np.int64).reshape(-1)
        idx_tiles = []
        for g in range(G):
            v = idx_c[g * GRP:(g + 1) * GRP].astype(np.int16)
            idx_tiles.append(np.tile(v.reshape(GRP // 16, 16).T, (8, 1)))
        idx = np.ascontiguousarray(np.concatenate(idx_tiles, axis=1))  # [128, G*32]
        rough = np.ascontiguousarray(
            np.asarray(top_rough_scores_batch[rows], dtype=np.float32).reshape(1, -1)
            + np.float32(np.asarray(bout).reshape(-1)[0]))
        in_maps.append({
            "amen": amen, "ment": ment, "w1bt": w1bt, "w1abt": w1abt,
            "w1at": w1at, "w1pw": w1pw, "b1t": b1t, "woutt": woutt,
            "pwt": pwt, "idx": idx, "rough": rough,
        })
    return in_maps, B, n_tab


_NC_CACHE = {}


def kernel_with_results(all_mentions, mentions_batch, pw_batch, top_indices_batch,
                        top_rough_scores_batch, W1, b1, Wout, bout, **run_kwargs):
    in_maps, B, n_tab = prep_inputs(
        all_mentions, mentions_batch, pw_batch, top_indices_batch,
        top_rough_scores_batch, W1, b1, Wout, bout)
    key = (B, n_tab)
    if key not in _NC_CACHE:
        _NC_CACHE[key] = build_nc(B, n_tab)
    nc = _NC_CACHE[key]
    res = run_bass_kernel_spmd(nc, in_maps, list(range(N_CORES)), **run_kwargs)
    scores = np.concatenate([np.asarray(r["out"]) for r in res.results], axis=0)
    batch = scores.shape[0]
    full = np.empty((batch, N_ANTS + 1), np.float32)
    full[:, 0] = EPS
    full[:, 1:] = scores
    return full, res


def kernel(**inputs) -> np.ndarray:
    out, _ = kernel_with_results(**inputs)
    return out


# revision 4
# speedup vs baseline: 1.0308x; 1.0308x over previous
"""Trainium2 Bass kernel for the AnaphoricityScorer (coref pairwise FFNN scorer).

Math (per batch row i, antecedent slot t):
    b  = all_mentions[top_indices[i, t]]                    # gathered mention
    pair = [a_i, b, a_i * b, pw[i, t]]                      # 3*1024 + 64 features
    h  = leaky_relu(pair @ W1.T + b1, 0.01)                 # 1024 hidden
    ffnn = h @ Wout.T + bout                                # scalar
    score = rough[i, t] + ffnn
    out = concat([eps_col, scores], axis=1)                 # [batch, 65]

Distribution: pure data parallel over the batch dim across 8 NeuronCores
(no collectives). all_mentions and FFNN weights are replicated.

Per-core algorithm (B = 128 batch rows -> 8192 pair rows, groups of 512):
  - The a-term a_i @ W1a.T is identical for all 64 antecedents of row i, so it
    is computed once per batch row in a prologue (ha = mentions @ W1a.T + b1)
    and broadcast-added into the pair-row PSUM with a stride-0 DVE add.
  - b rows arrive transposed (features on partitions) straight from HBM via
    dma_gather(transpose=True), which is exactly the matmul rhs layout.
  - a*b is built by a DVE multiply against a stride-0 broadcast of mentions^T.
  - One PSUM accumulation of 17 matmuls per (row-group, hidden-tile):
    8 K-tiles of W1b, 8 of W1ab, 1 of W1pw (K=64).
  - Lrelu on ScalarE evicts PSUM -> SBUF bf16; the Wout reduction is a
    K=128, M=1 matmul accumulated over the 8 hidden tiles.
  - bout + rough scores are added on DVE; one DMA out per core.

Everything is bf16 on the TensorEngine with fp32 PSUM accumulation.
"""

import sys

for _p in ("/opt/trn_rl_repo",):
    if _p not in sys.path:
        sys.path.append(_p)

import numpy as np
import ml_dtypes

import concourse.bacc as bacc
import concourse.mybir as mybir
from concourse.tile import TileContext
from concourse.bass_utils import run_bass_kernel_spmd

BF16 = mybir.dt.bfloat16
F32 = mybir.dt.float32
I16 = mybir.dt.int16

N_CORES = 8
EMB = 1024
HID = 1024
N_ANTS = 64
PW = 64
EPS = 1e-7
GRP = 512          # pair rows per group (= 8 batch rows)
ROWS_PER_GRP = 8   # batch rows per group


def build_nc(B: int, n_tab: int):
    """Build the per-core Bass graph. B = batch rows per core."""
    G = (B * N_ANTS) // GRP  # number of row groups
    FC = EMB // 128          # 8 feature k-tiles per 1024-feature block
    NT = HID // 128          # 8 hidden tiles

    nc = bacc.Bacc("TRN2")
    amen = nc.declare_dram_parameter("amen", [n_tab, EMB], BF16, isOutput=False)
    ment = nc.declare_dram_parameter("ment", [128, FC, B], BF16, isOutput=False)
    w1bt = nc.declare_dram_parameter("w1bt", [128, FC, HID], BF16, isOutput=False)
    w1abt = nc.declare_dram_parameter("w1abt", [128, FC, HID], BF16, isOutput=False)
    w1at = nc.declare_dram_parameter("w1at", [128, FC, HID], BF16, isOutput=False)
    w1pw = nc.declare_dram_parameter("w1pw", [PW, HID], BF16, isOutput=False)
    b1t = nc.declare_dram_parameter("b1t", [128, NT], F32, isOutput=False)
    woutt = nc.declare_dram_parameter("woutt", [128, NT], BF16, isOutput=False)
    pwt = nc.declare_dram_parameter("pwt", [PW, B * N_ANTS], BF16, isOutput=False)
    idx = nc.declare_dram_parameter("idx", [128, G * (GRP // 16)], I16, isOutput=False)
    rough = nc.declare_dram_parameter("rough", [1, B * N_ANTS], F32, isOutput=False)
    out = nc.declare_dram_parameter("out", [B, N_ANTS], F32, isOutput=True)

    with TileContext(nc) as tc:
        with (
            tc.tile_pool(name="const", bufs=1) as const,
            tc.tile_pool(name="btp", bufs=3) as btp,
            tc.tile_pool(name="abtp", bufs=3) as abtp,
            tc.tile_pool(name="htp", bufs=4) as htp,
            tc.tile_pool(name="psum", bufs=3, space="PSUM") as psum_pool,
            tc.tile_pool(name="psum_s", bufs=2, space="PSUM") as psum_s_pool,
        ):
            # ---- resident loads -------------------------------------------
            ment_t = const.tile([128, FC, B], BF16)
            nc.sync.dma_start(ment_t[:], ment[:, :, :])
            w1bt_t = const.tile([128, FC, HID], BF16)
            nc.sync.dma_start(w1bt_t[:], w1bt[:, :, :])
            w1abt_t = const.tile([128, FC, HID], BF16)
            nc.sync.dma_start(w1abt_t[:], w1abt[:, :, :])
            w1at_t = const.tile([128, FC, HID], BF16)
            nc.sync.dma_start(w1at_t[:], w1at[:, :, :])
            w1pw_t = const.tile([PW, HID], BF16)
            nc.sync.dma_start(w1pw_t[:], w1pw[:, :])
            b1t_t = const.tile([128, NT], F32)
            nc.sync.dma_start(b1t_t[:], b1t[:, :])
            woutt_t = const.tile([128, NT], BF16)
            nc.sync.dma_start(woutt_t[:], woutt[:, :])
            pwt_t = const.tile([PW, B * N_ANTS], BF16)
            nc.sync.dma_start(pwt_t[:], pwt[:, :])
            idx_t = const.tile([128, G * (GRP // 16)], I16)
            nc.sync.dma_start(idx_t[:], idx[:, :])
            rough_t = const.tile([1, B * N_ANTS], F32)
            nc.sync.dma_start(rough_t[:], rough[:, :])

            scores_t = const.tile([1, B * N_ANTS], F32)
            haT = const.tile([128, NT, B], F32)  # ha.T + b1, [hid%128, hidtile, row]

            # ---- prologue: ha.T = mentions @ W1a.T + b1 -------------------
            for nt in range(NT):
                pp = psum_pool.tile([128, B], F32)
                for fc in range(FC):
                    nc.tensor.matmul(
                        pp[:],
                        w1at_t[:, fc, nt * 128:(nt + 1) * 128],
                        ment_t[:, fc, :],
                        start=(fc == 0),
                        stop=(fc == FC - 1),
                    )
                nc.scalar.activation(
                    haT[:, nt, :], pp[:],
                    mybir.ActivationFunctionType.Identity,
                    bias=b1t_t[:, nt:nt + 1],
                )

            # ---- main loop over row groups --------------------------------
            # Software-pipelined emission: the gather + a*b multiplies for
            # group g+1 are emitted BEFORE group g's matmuls so the DVE
            # stream reaches them early, and each (g, nt) second-matmul is
            # deferred by one nt so its ht dependency never stalls PE.
            def produce_group(g):
                r0 = g * ROWS_PER_GRP
                bt = btp.tile([128, FC, GRP], BF16)
                nc.gpsimd.dma_gather(
                    bt[:], amen[:, :],
                    idx_t[:, g * (GRP // 16):(g + 1) * (GRP // 16)],
                    GRP, GRP, EMB, transpose=True,
                )
                abt = abtp.tile([128, FC, GRP], BF16)
                a_b = ment_t[:, :, r0:r0 + ROWS_PER_GRP]
                for fc in range(FC):
                    nc.vector.tensor_mul(
                        abt[:, fc, :].rearrange("p (a b) -> p a b", a=ROWS_PER_GRP),
                        bt[:, fc, :].rearrange("p (a b) -> p a b", a=ROWS_PER_GRP),
                        a_b[:, fc, :].unsqueeze(2).to_broadcast(
                            [128, ROWS_PER_GRP, N_ANTS]),
                    )
                return bt, abt

            pending = None  # deferred (ps_s, nt, ht) second-matmul
            tiles = {0: produce_group(0)}
            for g in range(G):
                r0 = g * ROWS_PER_GRP
                bt, abt = tiles.pop(g)
                if g + 1 < G:
                    tiles[g + 1] = produce_group(g + 1)
                ps_s = psum_s_pool.tile([1, GRP], F32)
                for nt in range(NT):
                    ps = psum_pool.tile([128, GRP], F32)
                    nsl = slice(nt * 128, (nt + 1) * 128)
                    for fc in range(FC):
                        nc.tensor.matmul(
                            ps[:], w1bt_t[:, fc, nsl], bt[:, fc, :],
                            start=(fc == 0), stop=False,
                        )
                    for fc in range(FC):
                        nc.tensor.matmul(
                            ps[:], w1abt_t[:, fc, nsl], abt[:, fc, :],
                            start=False, stop=False,
                        )
                    nc.tensor.matmul(
                        ps[:], w1pw_t[:, nsl],
                        pwt_t[:, g * GRP:(g + 1) * GRP],
                        start=False, stop=True,
                    )
                    # broadcast-add the per-batch-row a-term (incl. b1)
                    nc.vector.tensor_add(
                        ps[:].rearrange("p (a b) -> p a b", a=ROWS_PER_GRP),
                        ps[:].rearrange("p (a b) -> p a b", a=ROWS_PER_GRP),
                        haT[:, nt, r0:r0 + ROWS_PER_GRP].unsqueeze(2).to_broadcast(
                            [128, ROWS_PER_GRP, N_ANTS]),
                    )
                    ht = htp.tile([128, GRP], BF16)
                    nc.scalar.activation(
                        ht[:], ps[:],
                        mybir.ActivationFunctionType.Lrelu, alpha=0.01,
                    )
                    if pending is not None:
                        p_ps_s, p_nt, p_ht, p_g = pending
                        nc.tensor.matmul(
                            p_ps_s[:], woutt_t[:, p_nt:p_nt + 1], p_ht[:],
                            start=(p_nt == 0), stop=(p_nt == NT - 1),
                        )
                        if p_nt == NT - 1:
                            nc.vector.tensor_add(
                                scores_t[0:1, p_g * GRP:(p_g + 1) * GRP],
                                p_ps_s[0:1, :],
                                rough_t[0:1, p_g * GRP:(p_g + 1) * GRP],
                            )
                    pending = (ps_s, nt, ht, g)
            # flush the last deferred second-matmul
            p_ps_s, p_nt, p_ht, p_g = pending
            nc.tensor.matmul(
                p_ps_s[:], woutt_t[:, p_nt:p_nt + 1], p_ht[:],
                start=(p_nt == 0), stop=(p_nt == NT - 1),
            )
            nc.vector.tensor_add(
                scores_t[0:1, p_g * GRP:(p_g + 1) * GRP],
                p_ps_s[0:1, :],
                rough_t[0:1, p_g * GRP:(p_g + 1) * GRP],
            )

            nc.sync.dma_start(
                out[:, :].unsqueeze(0),
                scores_t[0:1, :].rearrange("p (a b) -> p a b", a=B),
            )

    nc.compile()
    return nc


def prep_inputs(all_mentions, mentions_batch, pw_batch, top_indices_batch,
                top_rough_scores_batch, W1, b1, Wout, bout, n_cores=N_CORES):
    """Host-side marshalling: shard over batch, cast/transpose into the
    layouts the kernel expects. Returns (in_maps, B, n_tab, bout_val)."""
    bf = ml_dtypes.bfloat16
    batch = mentions_batch.shape[0]
    B = batch // n_cores
    n_tab = all_mentions.shape[0]
    FC = EMB // 128
    NT = HID // 128
    G = (B * N_ANTS) // GRP

    amen = np.ascontiguousarray(all_mentions.astype(bf))

    def wt_block(Wcols):  # [1024, 1024] f32 block -> [128, FC, HID] bf16 (f on part)
        wt = Wcols.T.reshape(FC, 128, HID).transpose(1, 0, 2)
        return np.ascontiguousarray(wt.astype(bf))

    w1at = wt_block(W1[:, 0:EMB])
    w1bt = wt_block(W1[:, EMB:2 * EMB])
    w1abt = wt_block(W1[:, 2 * EMB:3 * EMB])
    w1pw = np.ascontiguousarray(W1[:, 3 * EMB:3 * EMB + PW].T.astype(bf))
    b1t = np.ascontiguousarray(b1.reshape(NT, 128).T.astype(np.float32))
    woutt = np.ascontiguousarray(Wout[0].reshape(NT, 128).T.astype(bf))

    in_maps = []
    for c in range(n_cores):
        rows = slice(c * B, (c + 1) * B)
        m_c = np.asarray(mentions_batch[rows], dtype=np.float32)       # [B, 1024]
        ment = np.ascontiguousarray(
            m_c.T.reshape(FC, 128, B).transpose(1, 0, 2).astype(bf))   # [128, FC, B]
        pw_c = np.asarray(pw_batch[rows], dtype=np.float32)            # [B, 64, 64]
        pwt = np.ascontiguousarray(pw_c.reshape(B * N_ANTS, PW).T.astype(bf))
        idx_c = np.asarray(top_indices_batch[rows]).astype(np.int64).reshape(-1)
        idx_tiles = []
        for g in range(G):
            v = idx_c[g * GRP:(g + 1) * GRP].astype(np.int16)
            idx_tiles.append(np.tile(v.reshape(GRP // 16, 16).T, (8, 1)))
        idx = np.ascontiguousarray(np.concatenate(idx_tiles, axis=1))  # [128, G*32]
        rough = np.ascontiguousarray(
            np.asarray(top_rough_scores_batch[rows], dtype=np.float32).reshape(1, -1)
            + np.float32(np.asarray(bout).reshape(-1)[0]))
        in_maps.append({
            "amen": amen, "ment": ment, "w1bt": w1bt, "w1abt": w1abt,
            "w1at": w1at, "w1pw": w1pw, "b1t": b1t, "woutt": woutt,
            "pwt": pwt, "idx": idx, "rough": rough,
        })
    return in_maps, B, n_tab


_NC_CACHE = {}


def kernel_with_results(all_mentions, mentions_batch, pw_batch, top_indices_batch,
                        top_rough_scores_batch, W1, b1, Wout, bout, **run_kwargs):
    in_maps, B, n_tab = prep_inputs(
        all_mentions, mentions_batch, pw_batch, top_indices_batch,
        top_rough_scores_batch, W1, b1, Wout, bout)
    key = (B, n_tab)
    if key not in _NC_CACHE:
        _NC_CACHE[key] = build_nc(B, n_tab)
    nc = _NC_CACHE[key]
    res = run_bass_kernel_spmd(nc, in_maps, list(range(N_CORES)), **run_kwargs)
    scores = np.concatenate([np.asarray(r["out"]) for r in res.results], axis=0)
    batch = scores.shape[0]
    full = np.empty((batch, N_ANTS + 1), np.float32)
    full[:, 0] = EPS
    full[:, 1:] = scores
    return full, res


def kernel(**inputs) -> np.ndarray:
    out, _ = kernel_with_results(**inputs)
    return out


# revision 7
# speedup vs baseline: 1.1869x; 1.1514x over previous
"""Trainium2 Bass kernel for the AnaphoricityScorer (coref pairwise FFNN scorer).

Math (per batch row i, antecedent slot t):
    b  = all_mentions[top_indices[i, t]]                    # gathered mention
    pair = [a_i, b, a_i * b, pw[i, t]]                      # 3*1024 + 64 features
    h  = leaky_relu(pair @ W1.T + b1, 0.01)                 # 1024 hidden
    ffnn = h @ Wout.T + bout                                # scalar
    score = rough[i, t] + ffnn
    out = concat([eps_col, scores], axis=1)                 # [batch, 65]

Distribution: pure data parallel over the batch dim across 8 NeuronCores
(no collectives). all_mentions and FFNN weights are replicated.

Per-core algorithm (B = 128 batch rows -> 8192 pair rows, groups of 512):
  - The a-term a_i @ W1a.T is identical for all 64 antecedents of row i, so it
    is computed once per batch row in a prologue (ha = mentions @ W1a.T + b1)
    and broadcast-added into the pair-row PSUM with a stride-0 DVE add.
  - b rows arrive transposed (features on partitions) straight from HBM via
    dma_gather(transpose=True), which is exactly the matmul rhs layout.
  - a*b is built by a DVE multiply against a stride-0 broadcast of mentions^T.
  - One PSUM accumulation of 17 matmuls per (row-group, hidden-tile):
    8 K-tiles of W1b, 8 of W1ab, 1 of W1pw (K=64).
  - Lrelu on ScalarE evicts PSUM -> SBUF bf16; the Wout reduction is a
    K=128, M=1 matmul accumulated over the 8 hidden tiles.
  - bout + rough scores are added on DVE; one DMA out per core.

Everything is bf16 on the TensorEngine with fp32 PSUM accumulation.
"""

import sys

for _p in ("/opt/trn_rl_repo",):
    if _p not in sys.path:
        sys.path.append(_p)

import numpy as np
import ml_dtypes

import concourse.bacc as bacc
import concourse.mybir as mybir
from concourse.tile import TileContext
from concourse.bass_utils import run_bass_kernel_spmd

BF16 = mybir.dt.bfloat16
F32 = mybir.dt.float32
I16 = mybir.dt.int16

N_CORES = 8
EMB = 1024
HID = 1024
N_ANTS = 64
PW = 64
EPS = 1e-7
GRP = 512          # pair rows per group (= 8 batch rows)
ROWS_PER_GRP = 8   # batch rows per group


def build_nc(B: int, n_tab: int):
    """Build the per-core Bass graph. B = batch rows per core."""
    G = (B * N_ANTS) // GRP  # number of row groups
    FC = EMB // 128          # 8 feature k-tiles per 1024-feature block
    NT = HID // 128          # 8 hidden tiles

    nc = bacc.Bacc("TRN2")
    amen = nc.declare_dram_parameter("amen", [n_tab, EMB], BF16, isOutput=False)
    ment = nc.declare_dram_parameter("ment", [128, FC, B], BF16, isOutput=False)
    w1bt = nc.declare_dram_parameter("w1bt", [128, FC, HID], BF16, isOutput=False)
    w1abt = nc.declare_dram_parameter("w1abt", [128, FC, HID], BF16, isOutput=False)
    w1at = nc.declare_dram_parameter("w1at", [128, FC, HID], BF16, isOutput=False)
    w1pw = nc.declare_dram_parameter("w1pw", [PW, HID], BF16, isOutput=False)
    b1t = nc.declare_dram_parameter("b1t", [128, NT], F32, isOutput=False)
    woutt = nc.declare_dram_parameter("woutt", [128, NT], BF16, isOutput=False)
    pwt = nc.declare_dram_parameter("pwt", [PW, B * N_ANTS], BF16, isOutput=False)
    idx = nc.declare_dram_parameter("idx", [128, G * (GRP // 16)], I16, isOutput=False)
    rough = nc.declare_dram_parameter("rough", [1, B * N_ANTS], F32, isOutput=False)
    out = nc.declare_dram_parameter("out", [B, N_ANTS], F32, isOutput=True)

    with TileContext(nc) as tc:
        with (
            tc.tile_pool(name="const", bufs=1) as const,
            tc.tile_pool(name="btp", bufs=5) as btp,
            tc.tile_pool(name="abtp", bufs=4) as abtp,
            tc.tile_pool(name="htp", bufs=4) as htp,
            tc.tile_pool(name="rpool", bufs=3) as rpool,
            tc.tile_pool(name="spool", bufs=3) as spool,
            tc.tile_pool(name="psum", bufs=3, space="PSUM") as psum_pool,
            tc.tile_pool(name="psum_s", bufs=2, space="PSUM") as psum_s_pool,
        ):
            # ---- resident loads -------------------------------------------
            ment_t = const.tile([128, FC, B], BF16)
            nc.sync.dma_start(ment_t[:], ment[:, :, :])
            w1bt_t = const.tile([128, FC, HID], BF16)
            nc.sync.dma_start(w1bt_t[:], w1bt[:, :, :])
            w1abt_t = const.tile([128, FC, HID], BF16)
            nc.sync.dma_start(w1abt_t[:], w1abt[:, :, :])
            w1at_t = const.tile([128, FC, HID], BF16)
            nc.sync.dma_start(w1at_t[:], w1at[:, :, :])
            w1pw_t = const.tile([PW, HID], BF16)
            nc.sync.dma_start(w1pw_t[:], w1pw[:, :])
            b1t_t = const.tile([128, NT], F32)
            nc.sync.dma_start(b1t_t[:], b1t[:, :])
            woutt_t = const.tile([128, NT], BF16)
            nc.sync.dma_start(woutt_t[:], woutt[:, :])
            pwt_t = const.tile([PW, B * N_ANTS], BF16)
            nc.sync.dma_start(pwt_t[:], pwt[:, :])
            idx_t = const.tile([128, G * (GRP // 16)], I16)
            nc.sync.dma_start(idx_t[:], idx[:, :])
            haT = const.tile([128, NT, B], F32)  # ha.T + b1, [hid%128, hidtile, row]

            # ---- prologue: ha.T = mentions @ W1a.T + b1 -------------------
            for nt in range(NT):
                pp = psum_pool.tile([128, B], F32)
                for fc in range(FC):
                    nc.tensor.matmul(
                        pp[:],
                        w1at_t[:, fc, nt * 128:(nt + 1) * 128],
                        ment_t[:, fc, :],
                        start=(fc == 0),
                        stop=(fc == FC - 1),
                    )
                nc.scalar.activation(
                    haT[:, nt, :], pp[:],
                    mybir.ActivationFunctionType.Identity,
                    bias=b1t_t[:, nt:nt + 1],
                )

            # ---- main loop over row groups --------------------------------
            # Software-pipelined emission: the gather + a*b multiplies for
            # group g+1 are emitted BEFORE group g's matmuls so the DVE
            # stream reaches them early, and each (g, nt) second-matmul is
            # deferred by one nt so its ht dependency never stalls PE.
            def produce_group(g):
                r0 = g * ROWS_PER_GRP
                rtile = rpool.tile([1, GRP], F32)
                nc.sync.dma_start(rtile[:], rough[0:1, g * GRP:(g + 1) * GRP])
                bt = btp.tile([128, FC, GRP], BF16)
                nc.gpsimd.dma_gather(
                    bt[:], amen[:, :],
                    idx_t[:, g * (GRP // 16):(g + 1) * (GRP // 16)],
                    GRP, GRP, EMB, transpose=True,
                )
                abt = abtp.tile([128, FC, GRP], BF16)
                a_b = ment_t[:, :, r0:r0 + ROWS_PER_GRP]
                for fc in range(FC):
                    nc.vector.tensor_mul(
                        abt[:, fc, :].rearrange("p (a b) -> p a b", a=ROWS_PER_GRP),
                        bt[:, fc, :].rearrange("p (a b) -> p a b", a=ROWS_PER_GRP),
                        a_b[:, fc, :].unsqueeze(2).to_broadcast(
                            [128, ROWS_PER_GRP, N_ANTS]),
                    )
                return bt, abt, rtile

            def finalize_group(p_ps_s, p_g, p_rtile):
                stile = spool.tile([1, GRP], F32)
                nc.vector.tensor_add(stile[:], p_ps_s[0:1, :], p_rtile[:])
                nc.sync.dma_start(
                    out[p_g * ROWS_PER_GRP:(p_g + 1) * ROWS_PER_GRP, :].unsqueeze(0),
                    stile[:].rearrange("p (r c) -> p r c", r=ROWS_PER_GRP),
                )

            pending = None  # deferred (ps_s, nt, ht) second-matmul
            tiles = {0: produce_group(0)}
            rtiles = {}
            for g in range(G):
                r0 = g * ROWS_PER_GRP
                bt, abt, rtiles[g] = tiles.pop(g)
                if g + 1 < G:
                    tiles[g + 1] = produce_group(g + 1)
                ps_s = psum_s_pool.tile([1, GRP], F32)
                for nt in range(NT):
                    ps = psum_pool.tile([128, GRP], F32)
                    nsl = slice(nt * 128, (nt + 1) * 128)
                    for fc in range(FC):
                        nc.tensor.matmul(
                            ps[:], w1bt_t[:, fc, nsl], bt[:, fc, :],
                            start=(fc == 0), stop=False,
                        )
                    for fc in range(FC):
                        nc.tensor.matmul(
                            ps[:], w1abt_t[:, fc, nsl], abt[:, fc, :],
                            start=False, stop=False,
                        )
                    nc.tensor.matmul(
                        ps[:], w1pw_t[:, nsl],
                        pwt_t[:, g * GRP:(g + 1) * GRP],
                        start=False, stop=True,
                    )
                    # broadcast-add the per-batch-row a-term (incl. b1)
                    nc.vector.tensor_add(
                        ps[:].rearrange("p (a b) -> p a b", a=ROWS_PER_GRP),
                        ps[:].rearrange("p (a b) -> p a b", a=ROWS_PER_GRP),
                        haT[:, nt, r0:r0 + ROWS_PER_GRP].unsqueeze(2).to_broadcast(
                            [128, ROWS_PER_GRP, N_ANTS]),
                    )
                    ht = htp.tile([128, GRP], BF16)
                    nc.scalar.activation(
                        ht[:], ps[:],
                        mybir.ActivationFunctionType.Lrelu, alpha=0.01,
                    )
                    if pending is not None:
                        p_ps_s, p_nt, p_ht, p_g = pending
                        nc.tensor.matmul(
                            p_ps_s[:], woutt_t[:, p_nt:p_nt + 1], p_ht[:],
                            start=(p_nt == 0), stop=(p_nt == NT - 1),
                        )
                        if p_nt == NT - 1:
                            finalize_group(p_ps_s, p_g, rtiles.pop(p_g))
                    pending = (ps_s, nt, ht, g)
            # flush the last deferred second-matmul
            p_ps_s, p_nt, p_ht, p_g = pending
            nc.tensor.matmul(
                p_ps_s[:], woutt_t[:, p_nt:p_nt + 1], p_ht[:],
                start=(p_nt == 0), stop=(p_nt == NT - 1),
            )
            finalize_group(p_ps_s, p_g, rtiles.pop(p_g))

    nc.compile()
    return nc


def prep_inputs(all_mentions, mentions_batch, pw_batch, top_indices_batch,
                top_rough_scores_batch, W1, b1, Wout, bout, n_cores=N_CORES):
    """Host-side marshalling: shard over batch, cast/transpose into the
    layouts the kernel expects. Returns (in_maps, B, n_tab, bout_val)."""
    bf = ml_dtypes.bfloat16
    batch = mentions_batch.shape[0]
    B = batch // n_cores
    n_tab = all_mentions.shape[0]
    FC = EMB // 128
    NT = HID // 128
    G = (B * N_ANTS) // GRP

    amen = np.ascontiguousarray(all_mentions.astype(bf))

    def wt_block(Wcols):  # [1024, 1024] f32 block -> [128, FC, HID] bf16 (f on part)
        wt = Wcols.T.reshape(FC, 128, HID).transpose(1, 0, 2)
        return np.ascontiguousarray(wt.astype(bf))

    w1at = wt_block(W1[:, 0:EMB])
    w1bt = wt_block(W1[:, EMB:2 * EMB])
    w1abt = wt_block(W1[:, 2 * EMB:3 * EMB])
    w1pw = np.ascontiguousarray(W1[:, 3 * EMB:3 * EMB + PW].T.astype(bf))
    b1t = np.ascontiguousarray(b1.reshape(NT, 128).T.astype(np.float32))
    woutt = np.ascontiguousarray(Wout[0].reshape(NT, 128).T.astype(bf))

    in_maps = []
    for c in range(n_cores):
        rows = slice(c * B, (c + 1) * B)
        m_c = np.asarray(mentions_batch[rows], dtype=np.float32)       # [B, 1024]
        ment = np.ascontiguousarray(
            m_c.T.reshape(FC, 128, B).transpose(1, 0, 2).astype(bf))   # [128, FC, B]
        pw_c = np.asarray(pw_batch[rows], dtype=np.float32)            # [B, 64, 64]
        pwt = np.ascontiguousarray(pw_c.reshape(B * N_ANTS, PW).T.astype(bf))
        idx_c = np.asarray(top_indices_batch[rows]).astype(np.int64).reshape(-1)
        idx_tiles = []
        for g in range(G):
            v = idx_c[g * GRP:(g + 1) * GRP].astype(np.int16)
            idx_tiles.append(np.tile(v.reshape(GRP // 16, 16).T, (8, 1)))
        idx = np.ascontiguousarray(np.concatenate(idx_tiles, axis=1))  # [128, G*32]
        rough = np.ascontiguousarray(
            np.asarray(top_rough_scores_batch[rows], dtype=np.float32).reshape(1, -1)
            + np.float32(np.asarray(bout).reshape(-1)[0]))
        in_maps.append({
            "amen": amen, "ment": ment, "w1bt": w1bt, "w1abt": w1abt,
            "w1at": w1at, "w1pw": w1pw, "b1t": b1t, "woutt": woutt,
            "pwt": pwt, "idx": idx, "rough": rough,
        })
    return in_maps, B, n_tab


_NC_CACHE = {}


def kernel_with_results(all_mentions, mentions_batch, pw_batch, top_indices_batch,
                        top_rough_scores_batch, W1, b1, Wout, bout, **run_kwargs):
    in_maps, B, n_tab = prep_inputs(
        all_mentions, mentions_batch, pw_batch, top_indices_batch,
        top_rough_scores_batch, W1, b1, Wout, bout)
    key = (B, n_tab)
    if key not in _NC_CACHE:
        _NC_CACHE[key] = build_nc(B, n_tab)
    nc = _NC_CACHE[key]
    res = run_bass_kernel_spmd(nc, in_maps, list(range(N_CORES)), **run_kwargs)
    scores = np.concatenate([np.asarray(r["out"]) for r in res.results], axis=0)
    batch = scores.shape[0]
    full = np.empty((batch, N_ANTS + 1), np.float32)
    full[:, 0] = EPS
    full[:, 1:] = scores
    return full, res


def kernel(**inputs) -> np.ndarray:
    out, _ = kernel_with_results(**inputs)
    return out


# revision 8
# speedup vs baseline: 1.8171x; 1.5310x over previous
"""Trainium2 Bass kernel for the AnaphoricityScorer (coref pairwise FFNN scorer).

Math (per batch row i, antecedent slot t):
    b  = all_mentions[top_indices[i, t]]                    # gathered mention
    pair = [a_i, b, a_i * b, pw[i, t]]                      # 3*1024 + 64 features
    h  = leaky_relu(pair @ W1.T + b1, 0.01)                 # 1024 hidden
    ffnn = h @ Wout.T + bout                                # scalar
    score = rough[i, t] + ffnn
    out = concat([eps_col, scores], axis=1)                 # [batch, 65]

Distribution: pure data parallel over the batch dim across 8 NeuronCores
(no collectives). all_mentions and FFNN weights are replicated.

Per-core algorithm (B = 128 batch rows -> 8192 pair rows, groups of 512):
  - The a-term a_i @ W1a.T is identical for all 64 antecedents of row i, so it
    is computed once per batch row in a prologue (ha = mentions @ W1a.T + b1)
    and broadcast-added into the pair-row PSUM with a stride-0 DVE add.
  - b rows arrive transposed (features on partitions) straight from HBM via
    dma_gather(transpose=True), which is exactly the matmul rhs layout.
  - a*b is built by a DVE multiply against a stride-0 broadcast of mentions^T.
  - One PSUM accumulation of 17 matmuls per (row-group, hidden-tile):
    8 K-tiles of W1b, 8 of W1ab, 1 of W1pw (K=64).
  - Lrelu on ScalarE evicts PSUM -> SBUF bf16; the Wout reduction is a
    K=128, M=1 matmul accumulated over the 8 hidden tiles.
  - bout + rough scores are added on DVE; one DMA out per core.

Everything is bf16 on the TensorEngine with fp32 PSUM accumulation.
"""

import sys

for _p in ("/opt/trn_rl_repo",):
    if _p not in sys.path:
        sys.path.append(_p)

import numpy as np
import ml_dtypes

import concourse.bacc as bacc
import concourse.mybir as mybir
from concourse.tile import TileContext
from concourse.bass_utils import run_bass_kernel_spmd

BF16 = mybir.dt.bfloat16
F32 = mybir.dt.float32
I16 = mybir.dt.int16
FP8 = mybir.dt.float8e4

USE_FP8 = True       # b/ab blocks in fp8-e4m3 DoubleRow (2 k-tiles per matmul)
FP8_SCALE = 512.0    # weight pre-scale so 0.02-magnitude weights leave fp8 denormals

N_CORES = 8
EMB = 1024
HID = 1024
N_ANTS = 64
PW = 64
EPS = 1e-7
GRP = 512          # pair rows per group (= 8 batch rows)
ROWS_PER_GRP = 8   # batch rows per group


def build_nc(B: int, n_tab: int):
    """Build the per-core Bass graph. B = batch rows per core."""
    G = (B * N_ANTS) // GRP  # number of row groups
    FC = EMB // 128          # 8 feature k-tiles per 1024-feature block
    NT = HID // 128          # 8 hidden tiles

    nc = bacc.Bacc("TRN2")
    amen = nc.declare_dram_parameter("amen", [n_tab, EMB], BF16, isOutput=False)
    ment = nc.declare_dram_parameter("ment", [128, FC, B], BF16, isOutput=False)
    wdt = FP8 if USE_FP8 else BF16
    w1bt = nc.declare_dram_parameter("w1bt", [128, FC, HID], wdt, isOutput=False)
    w1abt = nc.declare_dram_parameter("w1abt", [128, FC, HID], wdt, isOutput=False)
    w1at = nc.declare_dram_parameter("w1at", [128, FC, HID], BF16, isOutput=False)
    w1pw = nc.declare_dram_parameter("w1pw", [PW, HID], BF16, isOutput=False)
    b1t = nc.declare_dram_parameter("b1t", [128, NT], F32, isOutput=False)
    woutt = nc.declare_dram_parameter("woutt", [128, NT], BF16, isOutput=False)
    pwt = nc.declare_dram_parameter("pwt", [PW, B * N_ANTS], BF16, isOutput=False)
    idx = nc.declare_dram_parameter("idx", [128, G * (GRP // 16)], I16, isOutput=False)
    rough = nc.declare_dram_parameter("rough", [1, B * N_ANTS], F32, isOutput=False)
    out = nc.declare_dram_parameter("out", [B, N_ANTS], F32, isOutput=True)

    with TileContext(nc) as tc:
        with (
            tc.tile_pool(name="const", bufs=1) as const,
            tc.tile_pool(name="btp", bufs=5) as btp,
            tc.tile_pool(name="abtp", bufs=4) as abtp,
            tc.tile_pool(name="bt8p", bufs=4) as bt8p,
            tc.tile_pool(name="htp", bufs=4) as htp,
            tc.tile_pool(name="rpool", bufs=3) as rpool,
            tc.tile_pool(name="spool", bufs=3) as spool,
            tc.tile_pool(name="psum", bufs=3, space="PSUM") as psum_pool,
            tc.tile_pool(name="psum_s", bufs=2, space="PSUM") as psum_s_pool,
        ):
            # ---- resident loads -------------------------------------------
            ment_t = const.tile([128, FC, B], BF16)
            nc.sync.dma_start(ment_t[:], ment[:, :, :])
            w1bt_t = const.tile([128, FC, HID], wdt)
            nc.sync.dma_start(w1bt_t[:], w1bt[:, :, :])
            w1abt_t = const.tile([128, FC, HID], wdt)
            nc.sync.dma_start(w1abt_t[:], w1abt[:, :, :])
            w1at_t = const.tile([128, FC, HID], BF16)
            nc.sync.dma_start(w1at_t[:], w1at[:, :, :])
            w1pw_t = const.tile([PW, HID], BF16)
            nc.sync.dma_start(w1pw_t[:], w1pw[:, :])
            b1t_t = const.tile([128, NT], F32)
            nc.sync.dma_start(b1t_t[:], b1t[:, :])
            woutt_t = const.tile([128, NT], BF16)
            nc.sync.dma_start(woutt_t[:], woutt[:, :])
            pwt_t = const.tile([PW, B * N_ANTS], BF16)
            nc.sync.dma_start(pwt_t[:], pwt[:, :])
            idx_t = const.tile([128, G * (GRP // 16)], I16)
            nc.sync.dma_start(idx_t[:], idx[:, :])
            haT = const.tile([128, NT, B], F32)  # ha.T + b1, [hid%128, hidtile, row]

            # ---- prologue: ha.T = mentions @ W1a.T + b1 -------------------
            for nt in range(NT):
                pp = psum_pool.tile([128, B], F32)
                for fc in range(FC):
                    nc.tensor.matmul(
                        pp[:],
                        w1at_t[:, fc, nt * 128:(nt + 1) * 128],
                        ment_t[:, fc, :],
                        start=(fc == 0),
                        stop=(fc == FC - 1),
                    )
                nc.scalar.activation(
                    haT[:, nt, :], pp[:],
                    mybir.ActivationFunctionType.Identity,
                    bias=b1t_t[:, nt:nt + 1],
                )

            # ---- main loop over row groups --------------------------------
            # Software-pipelined emission: the gather + a*b multiplies for
            # group g+1 are emitted BEFORE group g's matmuls so the DVE
            # stream reaches them early, and each (g, nt) second-matmul is
            # deferred by one nt so its ht dependency never stalls PE.
            def produce_group(g):
                r0 = g * ROWS_PER_GRP
                rtile = rpool.tile([1, GRP], F32)
                nc.sync.dma_start(rtile[:], rough[0:1, g * GRP:(g + 1) * GRP])
                bt = btp.tile([128, FC, GRP], BF16)
                nc.gpsimd.dma_gather(
                    bt[:], amen[:, :],
                    idx_t[:, g * (GRP // 16):(g + 1) * (GRP // 16)],
                    GRP, GRP, EMB, transpose=True,
                )
                abt = abtp.tile([128, FC, GRP], FP8 if USE_FP8 else BF16)
                a_b = ment_t[:, :, r0:r0 + ROWS_PER_GRP]
                for fc in range(FC):
                    nc.vector.tensor_mul(
                        abt[:, fc, :].rearrange("p (a b) -> p a b", a=ROWS_PER_GRP),
                        bt[:, fc, :].rearrange("p (a b) -> p a b", a=ROWS_PER_GRP),
                        a_b[:, fc, :].unsqueeze(2).to_broadcast(
                            [128, ROWS_PER_GRP, N_ANTS]),
                    )
                if USE_FP8:
                    bt8 = bt8p.tile([128, FC, GRP], FP8)
                    for fc in range(FC):
                        nc.scalar.activation(
                            bt8[:, fc, :], bt[:, fc, :],
                            mybir.ActivationFunctionType.Identity)
                    bt = bt8
                return bt, abt, rtile

            def finalize_group(p_ps_s, p_g, p_rtile):
                stile = spool.tile([1, GRP], F32)
                nc.vector.tensor_add(stile[:], p_ps_s[0:1, :], p_rtile[:])
                nc.sync.dma_start(
                    out[p_g * ROWS_PER_GRP:(p_g + 1) * ROWS_PER_GRP, :].unsqueeze(0),
                    stile[:].rearrange("p (r c) -> p r c", r=ROWS_PER_GRP),
                )

            pending = None  # deferred (ps_s, nt, ht) second-matmul
            tiles = {0: produce_group(0)}
            rtiles = {}
            for g in range(G):
                r0 = g * ROWS_PER_GRP
                bt, abt, rtiles[g] = tiles.pop(g)
                if g + 1 < G:
                    tiles[g + 1] = produce_group(g + 1)
                ps_s = psum_s_pool.tile([1, GRP], F32)
                for nt in range(NT):
                    ps = psum_pool.tile([128, GRP], F32)
                    nsl = slice(nt * 128, (nt + 1) * 128)
                    if USE_FP8:
                        for fc in range(0, FC, 2):
                            nc.tensor.matmul(
                                ps[:], w1bt_t[:, fc:fc + 2, nsl], bt[:, fc:fc + 2, :],
                                perf_mode=mybir.MatmulPerfMode.DoubleRow,
                                start=(fc == 0), stop=False,
                            )
                        for fc in range(0, FC, 2):
                            nc.tensor.matmul(
                                ps[:], w1abt_t[:, fc:fc + 2, nsl], abt[:, fc:fc + 2, :],
                                perf_mode=mybir.MatmulPerfMode.DoubleRow,
                                start=False, stop=False,
                            )
                    else:
                        for fc in range(FC):
                            nc.tensor.matmul(
                                ps[:], w1bt_t[:, fc, nsl], bt[:, fc, :],
                                start=(fc == 0), stop=False,
                            )
                        for fc in range(FC):
                            nc.tensor.matmul(
                                ps[:], w1abt_t[:, fc, nsl], abt[:, fc, :],
                                start=False, stop=False,
                            )
                    nc.tensor.matmul(
                        ps[:], w1pw_t[:, nsl],
                        pwt_t[:, g * GRP:(g + 1) * GRP],
                        start=False, stop=True,
                    )
                    # broadcast-add the per-batch-row a-term (incl. b1)
                    nc.vector.tensor_add(
                        ps[:].rearrange("p (a b) -> p a b", a=ROWS_PER_GRP),
                        ps[:].rearrange("p (a b) -> p a b", a=ROWS_PER_GRP),
                        haT[:, nt, r0:r0 + ROWS_PER_GRP].unsqueeze(2).to_broadcast(
                            [128, ROWS_PER_GRP, N_ANTS]),
                    )
                    ht = htp.tile([128, GRP], BF16)
                    nc.scalar.activation(
                        ht[:], ps[:],
                        mybir.ActivationFunctionType.Lrelu, alpha=0.01,
                        scale=(1.0 / FP8_SCALE) if USE_FP8 else 1.0,
                    )
                    if pending is not None:
                        p_ps_s, p_nt, p_ht, p_g = pending
                        nc.tensor.matmul(
                            p_ps_s[:], woutt_t[:, p_nt:p_nt + 1], p_ht[:],
                            start=(p_nt == 0), stop=(p_nt == NT - 1),
                        )
                        if p_nt == NT - 1:
                            finalize_group(p_ps_s, p_g, rtiles.pop(p_g))
                    pending = (ps_s, nt, ht, g)
            # flush the last deferred second-matmul
            p_ps_s, p_nt, p_ht, p_g = pending
            nc.tensor.matmul(
                p_ps_s[:], woutt_t[:, p_nt:p_nt + 1], p_ht[:],
                start=(p_nt == 0), stop=(p_nt == NT - 1),
            )
            finalize_group(p_ps_s, p_g, rtiles.pop(p_g))

    nc.compile()
    return nc


def prep_inputs(all_mentions, mentions_batch, pw_batch, top_indices_batch,
                top_rough_scores_batch, W1, b1, Wout, bout, n_cores=N_CORES):
    """Host-side marshalling: shard over batch, cast/transpose into the
    layouts the kernel expects. Returns (in_maps, B, n_tab, bout_val)."""
    bf = ml_dtypes.bfloat16
    batch = mentions_batch.shape[0]
    B = batch // n_cores
    n_tab = all_mentions.shape[0]
    FC = EMB // 128
    NT = HID // 128
    G = (B * N_ANTS) // GRP

    amen = np.ascontiguousarray(all_mentions.astype(bf))

    def wt_block(Wcols, scale=1.0, dtype=bf):
        # [1024, 1024] f32 block -> [128, FC, HID] (feature on partitions)
        wt = Wcols.T.reshape(FC, 128, HID).transpose(1, 0, 2) * scale
        if dtype is not bf:
            wt = np.clip(wt, -240.0, 240.0)
        return np.ascontiguousarray(wt.astype(dtype))

    S = FP8_SCALE if USE_FP8 else 1.0
    f8 = ml_dtypes.float8_e4m3
    wdt = f8 if USE_FP8 else bf
    w1at = wt_block(W1[:, 0:EMB], S)
    w1bt = wt_block(W1[:, EMB:2 * EMB], S, wdt)
    w1abt = wt_block(W1[:, 2 * EMB:3 * EMB], S, wdt)
    w1pw = np.ascontiguousarray((W1[:, 3 * EMB:3 * EMB + PW].T * S).astype(bf))
    b1t = np.ascontiguousarray((b1.reshape(NT, 128).T * S).astype(np.float32))
    woutt = np.ascontiguousarray(Wout[0].reshape(NT, 128).T.astype(bf))

    in_maps = []
    for c in range(n_cores):
        rows = slice(c * B, (c + 1) * B)
        m_c = np.asarray(mentions_batch[rows], dtype=np.float32)       # [B, 1024]
        ment = np.ascontiguousarray(
            m_c.T.reshape(FC, 128, B).transpose(1, 0, 2).astype(bf))   # [128, FC, B]
        pw_c = np.asarray(pw_batch[rows], dtype=np.float32)            # [B, 64, 64]
        pwt = np.ascontiguousarray(pw_c.reshape(B * N_ANTS, PW).T.astype(bf))
        idx_c = np.asarray(top_indices_batch[rows]).astype(np.int64).reshape(-1)
        idx_tiles = []
        for g in range(G):
            v = idx_c[g * GRP:(g + 1) * GRP].astype(np.int16)
            idx_tiles.append(np.tile(v.reshape(GRP // 16, 16).T, (8, 1)))
        idx = np.ascontiguousarray(np.concatenate(idx_tiles, axis=1))  # [128, G*32]
        rough = np.ascontiguousarray(
            np.asarray(top_rough_scores_batch[rows], dtype=np.float32).reshape(1, -1)
            + np.float32(np.asarray(bout).reshape(-1)[0]))
        in_maps.append({
            "amen": amen, "ment": ment, "w1bt": w1bt, "w1abt": w1abt,
            "w1at": w1at, "w1pw": w1pw, "b1t": b1t, "woutt": woutt,
            "pwt": pwt, "idx": idx, "rough": rough,
        })
    return in_maps, B, n_tab


_NC_CACHE = {}


def kernel_with_results(all_mentions, mentions_batch, pw_batch, top_indices_batch,
                        top_rough_scores_batch, W1, b1, Wout, bout, **run_kwargs):
    in_maps, B, n_tab = prep_inputs(
        all_mentions, mentions_batch, pw_batch, top_indices_batch,
        top_rough_scores_batch, W1, b1, Wout, bout)
    key = (B, n_tab)
    if key not in _NC_CACHE:
        _NC_CACHE[key] = build_nc(B, n_tab)
    nc = _NC_CACHE[key]
    res = run_bass_kernel_spmd(nc, in_maps, list(range(N_CORES)), **run_kwargs)
    scores = np.concatenate([np.asarray(r["out"]) for r in res.results], axis=0)
    batch = scores.shape[0]
    full = np.empty((batch, N_ANTS + 1), np.float32)
    full[:, 0] = EPS
    full[:, 1:] = scores
    return full, res


def kernel(**inputs) -> np.ndarray:
    out, _ = kernel_with_results(**inputs)
    return out


# revision 9
# speedup vs baseline: 1.9277x; 1.0608x over previous
"""Trainium2 Bass kernel for the AnaphoricityScorer (coref pairwise FFNN scorer).

Math (per batch row i, antecedent slot t):
    b  = all_mentions[top_indices[i, t]]                    # gathered mention
    pair = [a_i, b, a_i * b, pw[i, t]]                      # 3*1024 + 64 features
    h  = leaky_relu(pair @ W1.T + b1, 0.01)                 # 1024 hidden
    ffnn = h @ Wout.T + bout                                # scalar
    score = rough[i, t] + ffnn
    out = concat([eps_col, scores], axis=1)                 # [batch, 65]

Distribution: pure data parallel over the batch dim across 8 NeuronCores
(no collectives). all_mentions and FFNN weights are replicated.

Per-core algorithm (B = 128 batch rows -> 8192 pair rows, groups of 512):
  - The a-term a_i @ W1a.T is identical for all 64 antecedents of row i, so it
    is computed once per batch row in a prologue (ha = mentions @ W1a.T + b1)
    and broadcast-added into the pair-row PSUM with a stride-0 DVE add.
  - b rows arrive transposed (features on partitions) straight from HBM via
    dma_gather(transpose=True), which is exactly the matmul rhs layout.
  - a*b is built by a DVE multiply against a stride-0 broadcast of mentions^T.
  - One PSUM accumulation of 17 matmuls per (row-group, hidden-tile):
    8 K-tiles of W1b, 8 of W1ab, 1 of W1pw (K=64).
  - Lrelu on ScalarE evicts PSUM -> SBUF bf16; the Wout reduction is a
    K=128, M=1 matmul accumulated over the 8 hidden tiles.
  - bout + rough scores are added on DVE; one DMA out per core.

Everything is bf16 on the TensorEngine with fp32 PSUM accumulation.
"""

import sys

for _p in ("/opt/trn_rl_repo",):
    if _p not in sys.path:
        sys.path.append(_p)

import numpy as np
import ml_dtypes

import concourse.bacc as bacc
import concourse.mybir as mybir
from concourse.tile import TileContext
from concourse.bass_utils import run_bass_kernel_spmd

BF16 = mybir.dt.bfloat16
F32 = mybir.dt.float32
I16 = mybir.dt.int16
FP8 = mybir.dt.float8e4

USE_FP8 = True       # b/ab blocks in fp8-e4m3 DoubleRow (2 k-tiles per matmul)
FP8_SCALE = 512.0    # weight pre-scale so 0.02-magnitude weights leave fp8 denormals

N_CORES = 8
EMB = 1024
HID = 1024
N_ANTS = 64
PW = 64
EPS = 1e-7
GRP = 512          # pair rows per group (= 8 batch rows)
ROWS_PER_GRP = 8   # batch rows per group


def build_nc(B: int, n_tab: int):
    """Build the per-core Bass graph. B = batch rows per core."""
    G = (B * N_ANTS) // GRP  # number of row groups
    FC = EMB // 128          # 8 feature k-tiles per 1024-feature block
    NT = HID // 128          # 8 hidden tiles

    nc = bacc.Bacc("TRN2")
    amen = nc.declare_dram_parameter("amen", [n_tab, EMB], BF16, isOutput=False)
    ment = nc.declare_dram_parameter("ment", [128, FC, B], BF16, isOutput=False)
    wdt = FP8 if USE_FP8 else BF16
    w1bt = nc.declare_dram_parameter("w1bt", [128, FC, HID], wdt, isOutput=False)
    w1abt = nc.declare_dram_parameter("w1abt", [128, FC, HID], wdt, isOutput=False)
    w1at = nc.declare_dram_parameter("w1at", [128, FC, HID], BF16, isOutput=False)
    w1pw = nc.declare_dram_parameter("w1pw", [128, HID], BF16, isOutput=False)
    b1t = nc.declare_dram_parameter("b1t", [128, NT], F32, isOutput=False)
    woutt = nc.declare_dram_parameter("woutt", [128, NT], BF16, isOutput=False)
    pwt = nc.declare_dram_parameter("pwt", [128, B * N_ANTS], BF16, isOutput=False)
    idx = nc.declare_dram_parameter("idx", [128, G * (GRP // 16)], I16, isOutput=False)
    rough = nc.declare_dram_parameter("rough", [1, B * N_ANTS], F32, isOutput=False)
    out = nc.declare_dram_parameter("out", [B, N_ANTS], F32, isOutput=True)

    with TileContext(nc) as tc:
        with (
            tc.tile_pool(name="const", bufs=1) as const,
            tc.tile_pool(name="btp", bufs=5) as btp,
            tc.tile_pool(name="abtp", bufs=4) as abtp,
            tc.tile_pool(name="bt8p", bufs=4) as bt8p,
            tc.tile_pool(name="htp", bufs=4) as htp,
            tc.tile_pool(name="rpool", bufs=3) as rpool,
            tc.tile_pool(name="spool", bufs=3) as spool,
            tc.tile_pool(name="psum", bufs=3, space="PSUM") as psum_pool,
            tc.tile_pool(name="psum_s", bufs=2, space="PSUM") as psum_s_pool,
        ):
            # ---- resident loads (gather + prologue deps first) ------------
            idx_t = const.tile([128, G * (GRP // 16)], I16)
            nc.sync.dma_start(idx_t[:], idx[:, :])
            ment_t = const.tile([128, FC, B], BF16)
            nc.sync.dma_start(ment_t[:], ment[:, :, :])
            w1at_t = const.tile([128, FC, HID], BF16)
            nc.sync.dma_start(w1at_t[:], w1at[:, :, :])
            b1t_t = const.tile([128, NT], F32)
            nc.sync.dma_start(b1t_t[:], b1t[:, :])
            w1bt_t = const.tile([128, FC, HID], wdt)
            nc.sync.dma_start(w1bt_t[:], w1bt[:, :, :])
            w1abt_t = const.tile([128, FC, HID], wdt)
            nc.sync.dma_start(w1abt_t[:], w1abt[:, :, :])
            w1pw_t = const.tile([128, HID], BF16)
            nc.sync.dma_start(w1pw_t[:], w1pw[:, :])
            woutt_t = const.tile([128, NT], BF16)
            nc.sync.dma_start(woutt_t[:], woutt[:, :])
            pwt_t = const.tile([128, B * N_ANTS], BF16)
            nc.sync.dma_start(pwt_t[:], pwt[:, :])
            haT = const.tile([128, NT, B], F32)  # ha.T + b1, [hid%128, hidtile, row]

            # ---- prologue: ha.T = mentions @ W1a.T + b1 -------------------
            for nt in range(NT):
                pp = psum_pool.tile([128, B], F32)
                for fc in range(FC):
                    nc.tensor.matmul(
                        pp[:],
                        w1at_t[:, fc, nt * 128:(nt + 1) * 128],
                        ment_t[:, fc, :],
                        start=(fc == 0),
                        stop=(fc == FC - 1),
                    )
                nc.scalar.activation(
                    haT[:, nt, :], pp[:],
                    mybir.ActivationFunctionType.Identity,
                    bias=b1t_t[:, nt:nt + 1],
                )

            # ---- main loop over row groups --------------------------------
            # Software-pipelined emission: the gather + a*b multiplies for
            # group g+1 are emitted BEFORE group g's matmuls so the DVE
            # stream reaches them early, and each (g, nt) second-matmul is
            # deferred by one nt so its ht dependency never stalls PE.
            def produce_group(g):
                r0 = g * ROWS_PER_GRP
                rtile = rpool.tile([1, GRP], F32)
                nc.sync.dma_start(rtile[:], rough[0:1, g * GRP:(g + 1) * GRP])
                bt = btp.tile([128, FC, GRP], BF16)
                nc.gpsimd.dma_gather(
                    bt[:], amen[:, :],
                    idx_t[:, g * (GRP // 16):(g + 1) * (GRP // 16)],
                    GRP, GRP, EMB, transpose=True,
                )
                abt = abtp.tile([128, FC, GRP], FP8 if USE_FP8 else BF16)
                a_b = ment_t[:, :, r0:r0 + ROWS_PER_GRP]
                for fc in range(FC):
                    nc.vector.tensor_mul(
                        abt[:, fc, :].rearrange("p (a b) -> p a b", a=ROWS_PER_GRP),
                        bt[:, fc, :].rearrange("p (a b) -> p a b", a=ROWS_PER_GRP),
                        a_b[:, fc, :].unsqueeze(2).to_broadcast(
                            [128, ROWS_PER_GRP, N_ANTS]),
                    )
                if USE_FP8:
                    bt8 = bt8p.tile([128, FC, GRP], FP8)
                    for fc in range(FC):
                        nc.scalar.activation(
                            bt8[:, fc, :], bt[:, fc, :],
                            mybir.ActivationFunctionType.Identity)
                    bt = bt8
                return bt, abt, rtile

            def finalize_group(p_ps_s, p_g, p_rtile):
                stile = spool.tile([1, GRP], F32)
                nc.vector.tensor_add(stile[:], p_ps_s[0:1, :], p_rtile[:])
                nc.sync.dma_start(
                    out[p_g * ROWS_PER_GRP:(p_g + 1) * ROWS_PER_GRP, :].unsqueeze(0),
                    stile[:].rearrange("p (r c) -> p r c", r=ROWS_PER_GRP),
                )

            pending = None  # deferred (ps_s, nt, ht) second-matmul
            tiles = {0: produce_group(0)}
            rtiles = {}
            for g in range(G):
                r0 = g * ROWS_PER_GRP
                bt, abt, rtiles[g] = tiles.pop(g)
                if g + 1 < G:
                    tiles[g + 1] = produce_group(g + 1)
                ps_s = psum_s_pool.tile([1, GRP], F32)
                for nt in range(NT):
                    ps = psum_pool.tile([128, GRP], F32)
                    nsl = slice(nt * 128, (nt + 1) * 128)
                    if USE_FP8:
                        for fc in range(0, FC, 2):
                            nc.tensor.matmul(
                                ps[:], w1bt_t[:, fc:fc + 2, nsl], bt[:, fc:fc + 2, :],
                                perf_mode=mybir.MatmulPerfMode.DoubleRow,
                                start=(fc == 0), stop=False,
                            )
                        for fc in range(0, FC, 2):
                            nc.tensor.matmul(
                                ps[:], w1abt_t[:, fc:fc + 2, nsl], abt[:, fc:fc + 2, :],
                                perf_mode=mybir.MatmulPerfMode.DoubleRow,
                                start=False, stop=False,
                            )
                    else:
                        for fc in range(FC):
                            nc.tensor.matmul(
                                ps[:], w1bt_t[:, fc, nsl], bt[:, fc, :],
                                start=(fc == 0), stop=False,
                            )
                        for fc in range(FC):
                            nc.tensor.matmul(
                                ps[:], w1abt_t[:, fc, nsl], abt[:, fc, :],
                                start=False, stop=False,
                            )
                    nc.tensor.matmul(
                        ps[:], w1pw_t[:, nsl],
                        pwt_t[:, g * GRP:(g + 1) * GRP],
                        start=False, stop=True,
                    )
                    # broadcast-add the per-batch-row a-term (incl. b1)
                    nc.vector.tensor_add(
                        ps[:].rearrange("p (a b) -> p a b", a=ROWS_PER_GRP),
                        ps[:].rearrange("p (a b) -> p a b", a=ROWS_PER_GRP),
                        haT[:, nt, r0:r0 + ROWS_PER_GRP].unsqueeze(2).to_broadcast(
                            [128, ROWS_PER_GRP, N_ANTS]),
                    )
                    ht = htp.tile([128, GRP], BF16)
                    nc.scalar.activation(
                        ht[:], ps[:],
                        mybir.ActivationFunctionType.Lrelu, alpha=0.01,
                        scale=(1.0 / FP8_SCALE) if USE_FP8 else 1.0,
                    )
                    if pending is not None:
                        p_ps_s, p_nt, p_ht, p_g = pending
                        nc.tensor.matmul(
                            p_ps_s[:], woutt_t[:, p_nt:p_nt + 1], p_ht[:],
                            start=(p_nt == 0), stop=(p_nt == NT - 1),
                        )
                        if p_nt == NT - 1:
                            finalize_group(p_ps_s, p_g, rtiles.pop(p_g))
                    pending = (ps_s, nt, ht, g)
            # flush the last deferred second-matmul
            p_ps_s, p_nt, p_ht, p_g = pending
            nc.tensor.matmul(
                p_ps_s[:], woutt_t[:, p_nt:p_nt + 1], p_ht[:],
                start=(p_nt == 0), stop=(p_nt == NT - 1),
            )
            finalize_group(p_ps_s, p_g, rtiles.pop(p_g))

    nc.compile()
    return nc


def prep_inputs(all_mentions, mentions_batch, pw_batch, top_indices_batch,
                top_rough_scores_batch, W1, b1, Wout, bout, n_cores=N_CORES):
    """Host-side marshalling: shard over batch, cast/transpose into the
    layouts the kernel expects. Returns (in_maps, B, n_tab, bout_val)."""
    bf = ml_dtypes.bfloat16
    batch = mentions_batch.shape[0]
    B = batch // n_cores
    n_tab = all_mentions.shape[0]
    FC = EMB // 128
    NT = HID // 128
    G = (B * N_ANTS) // GRP

    amen = np.ascontiguousarray(all_mentions.astype(bf))

    def wt_block(Wcols, scale=1.0, dtype=bf):
        # [1024, 1024] f32 block -> [128, FC, HID] (feature on partitions)
        wt = Wcols.T.reshape(FC, 128, HID).transpose(1, 0, 2) * scale
        if dtype is not bf:
            wt = np.clip(wt, -240.0, 240.0)
        return np.ascontiguousarray(wt.astype(dtype))

    S = FP8_SCALE if USE_FP8 else 1.0
    f8 = ml_dtypes.float8_e4m3
    wdt = f8 if USE_FP8 else bf
    w1at = wt_block(W1[:, 0:EMB], S)
    w1bt = wt_block(W1[:, EMB:2 * EMB], S, wdt)
    w1abt = wt_block(W1[:, 2 * EMB:3 * EMB], S, wdt)
    w1pw = np.zeros((128, HID), dtype=bf)
    w1pw[:PW] = (W1[:, 3 * EMB:3 * EMB + PW].T * S).astype(bf)
    b1t = np.ascontiguousarray((b1.reshape(NT, 128).T * S).astype(np.float32))
    woutt = np.ascontiguousarray(Wout[0].reshape(NT, 128).T.astype(bf))

    in_maps = []
    for c in range(n_cores):
        rows = slice(c * B, (c + 1) * B)
        m_c = np.asarray(mentions_batch[rows], dtype=np.float32)       # [B, 1024]
        ment = np.ascontiguousarray(
            m_c.T.reshape(FC, 128, B).transpose(1, 0, 2).astype(bf))   # [128, FC, B]
        pw_c = np.asarray(pw_batch[rows], dtype=np.float32)            # [B, 64, 64]
        pwt = np.zeros((128, B * N_ANTS), dtype=bf)
        pwt[:PW] = pw_c.reshape(B * N_ANTS, PW).T.astype(bf)
        idx_c = np.asarray(top_indices_batch[rows]).astype(np.int64).reshape(-1)
        idx_tiles = []
        for g in range(G):
            v = idx_c[g * GRP:(g + 1) * GRP].astype(np.int16)
            idx_tiles.append(np.tile(v.reshape(GRP // 16, 16).T, (8, 1)))
        idx = np.ascontiguousarray(np.concatenate(idx_tiles, axis=1))  # [128, G*32]
        rough = np.ascontiguousarray(
            np.asarray(top_rough_scores_batch[rows], dtype=np.float32).reshape(1, -1)
            + np.float32(np.asarray(bout).reshape(-1)[0]))
        in_maps.append({
            "amen": amen, "ment": ment, "w1bt": w1bt, "w1abt": w1abt,
            "w1at": w1at, "w1pw": w1pw, "b1t": b1t, "woutt": woutt,
            "pwt": pwt, "idx": idx, "rough": rough,
        })
    return in_maps, B, n_tab


_NC_CACHE = {}


def kernel_with_results(all_mentions, mentions_batch, pw_batch, top_indices_batch,
                        top_rough_scores_batch, W1, b1, Wout, bout, **run_kwargs):
    in_maps, B, n_tab = prep_inputs(
        all_mentions, mentions_batch, pw_batch, top_indices_batch,
        top_rough_scores_batch, W1, b1, Wout, bout)
    key = (B, n_tab)
    if key not in _NC_CACHE:
        _NC_CACHE[key] = build_nc(B, n_tab)
    nc = _NC_CACHE[key]
    res = run_bass_kernel_spmd(nc, in_maps, list(range(N_CORES)), **run_kwargs)
    scores = np.concatenate([np.asarray(r["out"]) for r in res.results], axis=0)
    batch = scores.shape[0]
    full = np.empty((batch, N_ANTS + 1), np.float32)
    full[:, 0] = EPS
    full[:, 1:] = scores
    return full, res


def kernel(**inputs) -> np.ndarray:
    out, _ = kernel_with_results(**inputs)
    return out


# revision 11
# speedup vs baseline: 2.1381x; 1.1092x over previous
"""Trainium2 Bass kernel for the AnaphoricityScorer (coref pairwise FFNN scorer).

Math (per batch row i, antecedent slot t):
    b  = all_mentions[top_indices[i, t]]                    # gathered mention
    pair = [a_i, b, a_i * b, pw[i, t]]                      # 3*1024 + 64 features
    h  = leaky_relu(pair @ W1.T + b1, 0.01)                 # 1024 hidden
    ffnn = h @ Wout.T + bout                                # scalar
    score = rough[i, t] + ffnn
    out = concat([eps_col, scores], axis=1)                 # [batch, 65]

Distribution: pure data parallel over the batch dim across 8 NeuronCores
(no collectives). all_mentions and FFNN weights are replicated.

Per-core algorithm (B = 128 batch rows -> 8192 pair rows, groups of 512):
  - The a-term a_i @ W1a.T is identical for all 64 antecedents of row i, so it
    is computed once per batch row in a prologue (ha = mentions @ W1a.T + b1)
    and broadcast-added into the pair-row PSUM with a stride-0 DVE add.
  - b rows arrive transposed (features on partitions) straight from HBM via
    dma_gather(transpose=True), which is exactly the matmul rhs layout.
  - a*b is built by a DVE multiply against a stride-0 broadcast of mentions^T.
  - One PSUM accumulation of 17 matmuls per (row-group, hidden-tile):
    8 K-tiles of W1b, 8 of W1ab, 1 of W1pw (K=64).
  - Lrelu on ScalarE evicts PSUM -> SBUF bf16; the Wout reduction is a
    K=128, M=1 matmul accumulated over the 8 hidden tiles.
  - bout + rough scores are added on DVE; one DMA out per core.

Everything is bf16 on the TensorEngine with fp32 PSUM accumulation.
"""

import sys

for _p in ("/opt/trn_rl_repo",):
    if _p not in sys.path:
        sys.path.append(_p)

import numpy as np
import ml_dtypes

import concourse.bacc as bacc
import concourse.mybir as mybir
from concourse.tile import TileContext
from concourse.bass_utils import run_bass_kernel_spmd

BF16 = mybir.dt.bfloat16
F32 = mybir.dt.float32
I16 = mybir.dt.int16
FP8 = mybir.dt.float8e4

USE_FP8 = True       # b/ab blocks in fp8-e4m3 DoubleRow (2 k-tiles per matmul)
FP8_SCALE = 512.0    # weight pre-scale so 0.02-magnitude weights leave fp8 denormals

N_CORES = 8
EMB = 1024
HID = 1024
N_ANTS = 64
PW = 64
EPS = 1e-7
GRP = 512          # pair rows per group (= 8 batch rows)
ROWS_PER_GRP = 8   # batch rows per group


def build_nc(B: int, n_tab: int):
    """Build the per-core Bass graph. B = batch rows per core."""
    G = (B * N_ANTS) // GRP  # number of row groups
    FC = EMB // 128          # 8 feature k-tiles per 1024-feature block
    NT = HID // 128          # 8 hidden tiles

    nc = bacc.Bacc("TRN2")
    amen = nc.declare_dram_parameter("amen", [n_tab, EMB], BF16, isOutput=False)
    ment = nc.declare_dram_parameter("ment", [128, FC, B], BF16, isOutput=False)
    wdt = FP8 if USE_FP8 else BF16
    w1bt = nc.declare_dram_parameter("w1bt", [128, FC, HID], wdt, isOutput=False)
    w1abt = nc.declare_dram_parameter("w1abt", [128, FC, HID], wdt, isOutput=False)
    w1at = nc.declare_dram_parameter("w1at", [128, FC, HID], BF16, isOutput=False)
    w1pw = nc.declare_dram_parameter("w1pw", [128, HID], BF16, isOutput=False)
    b1t = nc.declare_dram_parameter("b1t", [128, NT], F32, isOutput=False)
    woutt = nc.declare_dram_parameter("woutt", [128, NT], BF16, isOutput=False)
    pwt = nc.declare_dram_parameter("pwt", [128, B * N_ANTS], BF16, isOutput=False)
    idx = nc.declare_dram_parameter("idx", [128, G * (GRP // 16)], I16, isOutput=False)
    rough = nc.declare_dram_parameter("rough", [1, B * N_ANTS], F32, isOutput=False)
    out = nc.declare_dram_parameter("out", [B, N_ANTS], F32, isOutput=True)

    with TileContext(nc) as tc:
        with (
            tc.tile_pool(name="const", bufs=1) as const,
            tc.tile_pool(name="btp", bufs=5) as btp,
            tc.tile_pool(name="abtp", bufs=4) as abtp,
            tc.tile_pool(name="bt8p", bufs=4) as bt8p,
            tc.tile_pool(name="htp", bufs=10) as htp,
            tc.tile_pool(name="rpool", bufs=3) as rpool,
            tc.tile_pool(name="spool", bufs=2) as spool,
            tc.tile_pool(name="psum", bufs=3, space="PSUM") as psum_pool,
            tc.tile_pool(name="psum_s", bufs=2, space="PSUM") as psum_s_pool,
        ):
            # ---- resident loads (gather + prologue deps first) ------------
            idx_t = const.tile([128, G * (GRP // 16)], I16)
            nc.sync.dma_start(idx_t[:], idx[:, :])
            ment_t = const.tile([128, FC, B], BF16)
            nc.sync.dma_start(ment_t[:], ment[:, :, :])
            w1at_t = const.tile([128, FC, HID], BF16)
            nc.sync.dma_start(w1at_t[:], w1at[:, :, :])
            b1t_t = const.tile([128, NT], F32)
            nc.sync.dma_start(b1t_t[:], b1t[:, :])
            w1bt_t = const.tile([128, FC, HID], wdt)
            nc.sync.dma_start(w1bt_t[:], w1bt[:, :, :])
            w1abt_t = const.tile([128, FC, HID], wdt)
            nc.sync.dma_start(w1abt_t[:], w1abt[:, :, :])
            w1pw_t = const.tile([128, HID], BF16)
            nc.sync.dma_start(w1pw_t[:], w1pw[:, :])
            woutt_t = const.tile([128, NT], BF16)
            nc.sync.dma_start(woutt_t[:], woutt[:, :])
            pwt_t = const.tile([128, B * N_ANTS], BF16)
            nc.sync.dma_start(pwt_t[:], pwt[:, :])
            haT = const.tile([128, NT, B], F32)  # ha.T + b1, [hid%128, hidtile, row]

            # ---- prologue: ha.T = mentions @ W1a.T + b1 -------------------
            for nt in range(NT):
                pp = psum_pool.tile([128, B], F32)
                for fc in range(FC):
                    nc.tensor.matmul(
                        pp[:],
                        w1at_t[:, fc, nt * 128:(nt + 1) * 128],
                        ment_t[:, fc, :],
                        start=(fc == 0),
                        stop=(fc == FC - 1),
                    )
                nc.scalar.activation(
                    haT[:, nt, :], pp[:],
                    mybir.ActivationFunctionType.Identity,
                    bias=b1t_t[:, nt:nt + 1],
                )

            # ---- main loop over row groups --------------------------------
            # Software-pipelined emission: the gather + a*b multiplies for
            # group g+1 are emitted BEFORE group g's matmuls so the DVE
            # stream reaches them early, and each (g, nt) second-matmul is
            # deferred by one nt so its ht dependency never stalls PE.
            def produce_group(g):
                r0 = g * ROWS_PER_GRP
                rtile = rpool.tile([1, GRP], F32)
                nc.sync.dma_start(rtile[:], rough[0:1, g * GRP:(g + 1) * GRP])
                bt = btp.tile([128, FC, GRP], BF16)
                nc.gpsimd.dma_gather(
                    bt[:], amen[:, :],
                    idx_t[:, g * (GRP // 16):(g + 1) * (GRP // 16)],
                    GRP, GRP, EMB, transpose=True,
                )
                abt = abtp.tile([128, FC, GRP], FP8 if USE_FP8 else BF16)
                a_b = ment_t[:, :, r0:r0 + ROWS_PER_GRP]
                for fc in range(FC):
                    nc.vector.tensor_mul(
                        abt[:, fc, :].rearrange("p (a b) -> p a b", a=ROWS_PER_GRP),
                        bt[:, fc, :].rearrange("p (a b) -> p a b", a=ROWS_PER_GRP),
                        a_b[:, fc, :].unsqueeze(2).to_broadcast(
                            [128, ROWS_PER_GRP, N_ANTS]),
                    )
                if USE_FP8:
                    bt8 = bt8p.tile([128, FC, GRP], FP8)
                    for fc in range(FC):
                        nc.scalar.activation(
                            bt8[:, fc, :], bt[:, fc, :],
                            mybir.ActivationFunctionType.Identity)
                    bt = bt8
                return bt, abt, rtile

            def emit_batch(ps4, hts, nts, start):
                # 4 M=1 matmuls packed into distinct PE column groups -- they
                # execute concurrently in the array (one per 32-col strip)
                for nt_i, ht_i in zip(nts, hts):
                    j = nt_i % 4
                    nc.tensor.matmul(
                        ps4[32 * j:32 * j + 1, :], woutt_t[:, nt_i:nt_i + 1],
                        ht_i[:], tile_position=(0, 32 * j),
                        start=start, stop=not start,
                    )

            def finalize_group(ps4, p_g, p_rtile):
                # DVE may read at most one PSUM operand per op: chain the four
                # column-group partial rows through SBUF
                t1 = spool.tile([1, GRP], F32)
                nc.vector.tensor_add(t1[:], ps4[0:1, :], p_rtile[:])
                t2 = spool.tile([1, GRP], F32)
                nc.vector.tensor_add(t2[:], ps4[32:33, :], t1[:])
                t3 = spool.tile([1, GRP], F32)
                nc.vector.tensor_add(t3[:], ps4[64:65, :], t2[:])
                stile = spool.tile([1, GRP], F32)
                nc.vector.tensor_add(stile[:], ps4[96:97, :], t3[:])
                nc.sync.dma_start(
                    out[p_g * ROWS_PER_GRP:(p_g + 1) * ROWS_PER_GRP, :].unsqueeze(0),
                    stile[:].rearrange("p (r c) -> p r c", r=ROWS_PER_GRP),
                )

            tiles = {0: produce_group(0)}
            rtiles = {}
            prev_group = None  # (g, ps4, hts) awaiting its second batch
            for g in range(G):
                r0 = g * ROWS_PER_GRP
                bt, abt, rtiles[g] = tiles.pop(g)
                if g + 1 < G:
                    tiles[g + 1] = produce_group(g + 1)
                hts = []
                ps4 = None
                for nt in range(NT):
                    ps = psum_pool.tile([128, GRP], F32)
                    nsl = slice(nt * 128, (nt + 1) * 128)
                    if USE_FP8:
                        for fc in range(0, FC, 2):
                            nc.tensor.matmul(
                                ps[:], w1bt_t[:, fc:fc + 2, nsl], bt[:, fc:fc + 2, :],
                                perf_mode=mybir.MatmulPerfMode.DoubleRow,
                                start=(fc == 0), stop=False,
                            )
                        for fc in range(0, FC, 2):
                            nc.tensor.matmul(
                                ps[:], w1abt_t[:, fc:fc + 2, nsl], abt[:, fc:fc + 2, :],
                                perf_mode=mybir.MatmulPerfMode.DoubleRow,
                                start=False, stop=False,
                            )
                    else:
                        for fc in range(FC):
                            nc.tensor.matmul(
                                ps[:], w1bt_t[:, fc, nsl], bt[:, fc, :],
                                start=(fc == 0), stop=False,
                            )
                        for fc in range(FC):
                            nc.tensor.matmul(
                                ps[:], w1abt_t[:, fc, nsl], abt[:, fc, :],
                                start=False, stop=False,
                            )
                    nc.tensor.matmul(
                        ps[:], w1pw_t[:, nsl],
                        pwt_t[:, g * GRP:(g + 1) * GRP],
                        start=False, stop=True,
                    )
                    # broadcast-add the per-batch-row a-term (incl. b1)
                    nc.vector.tensor_add(
                        ps[:].rearrange("p (a b) -> p a b", a=ROWS_PER_GRP),
                        ps[:].rearrange("p (a b) -> p a b", a=ROWS_PER_GRP),
                        haT[:, nt, r0:r0 + ROWS_PER_GRP].unsqueeze(2).to_broadcast(
                            [128, ROWS_PER_GRP, N_ANTS]),
                    )
                    ht = htp.tile([128, GRP], BF16)
                    nc.scalar.activation(
                        ht[:], ps[:],
                        mybir.ActivationFunctionType.Lrelu, alpha=0.01,
                        scale=(1.0 / FP8_SCALE) if USE_FP8 else 1.0,
                    )
                    hts.append(ht)
                    if nt == 1 and prev_group is not None:
                        p_g, p_ps4, p_hts = prev_group
                        emit_batch(p_ps4, p_hts[4:8], range(4, 8), start=False)
                        finalize_group(p_ps4, p_g, rtiles.pop(p_g))
                        prev_group = None
                    if nt == 5:
                        ps4 = psum_s_pool.tile([128, GRP], F32)
                        emit_batch(ps4, hts[0:4], range(0, 4), start=True)
                prev_group = (g, ps4, hts)
            # flush the last group's second batch
            p_g, p_ps4, p_hts = prev_group
            emit_batch(p_ps4, p_hts[4:8], range(4, 8), start=False)
            finalize_group(p_ps4, p_g, rtiles.pop(p_g))

    nc.compile()
    return nc


def prep_inputs(all_mentions, mentions_batch, pw_batch, top_indices_batch,
                top_rough_scores_batch, W1, b1, Wout, bout, n_cores=N_CORES):
    """Host-side marshalling: shard over batch, cast/transpose into the
    layouts the kernel expects. Returns (in_maps, B, n_tab, bout_val)."""
    bf = ml_dtypes.bfloat16
    batch = mentions_batch.shape[0]
    B = batch // n_cores
    n_tab = all_mentions.shape[0]
    FC = EMB // 128
    NT = HID // 128
    G = (B * N_ANTS) // GRP

    amen = np.ascontiguousarray(all_mentions.astype(bf))

    def wt_block(Wcols, scale=1.0, dtype=bf):
        # [1024, 1024] f32 block -> [128, FC, HID] (feature on partitions)
        wt = Wcols.T.reshape(FC, 128, HID).transpose(1, 0, 2) * scale
        if dtype is not bf:
            wt = np.clip(wt, -240.0, 240.0)
        return np.ascontiguousarray(wt.astype(dtype))

    S = FP8_SCALE if USE_FP8 else 1.0
    f8 = ml_dtypes.float8_e4m3
    wdt = f8 if USE_FP8 else bf
    w1at = wt_block(W1[:, 0:EMB], S)
    w1bt = wt_block(W1[:, EMB:2 * EMB], S, wdt)
    w1abt = wt_block(W1[:, 2 * EMB:3 * EMB], S, wdt)
    w1pw = np.zeros((128, HID), dtype=bf)
    w1pw[:PW] = (W1[:, 3 * EMB:3 * EMB + PW].T * S).astype(bf)
    b1t = np.ascontiguousarray((b1.reshape(NT, 128).T * S).astype(np.float32))
    woutt = np.ascontiguousarray(Wout[0].reshape(NT, 128).T.astype(bf))

    in_maps = []
    for c in range(n_cores):
        rows = slice(c * B, (c + 1) * B)
        m_c = np.asarray(mentions_batch[rows], dtype=np.float32)       # [B, 1024]
        ment = np.ascontiguousarray(
            m_c.T.reshape(FC, 128, B).transpose(1, 0, 2).astype(bf))   # [128, FC, B]
        pw_c = np.asarray(pw_batch[rows], dtype=np.float32)            # [B, 64, 64]
        pwt = np.zeros((128, B * N_ANTS), dtype=bf)
        pwt[:PW] = pw_c.reshape(B * N_ANTS, PW).T.astype(bf)
        idx_c = np.asarray(top_indices_batch[rows]).astype(np.int64).reshape(-1)
        idx_tiles = []
        for g in range(G):
            v = idx_c[g * GRP:(g + 1) * GRP].astype(np.int16)
            idx_tiles.append(np.tile(v.reshape(GRP // 16, 16).T, (8, 1)))
        idx = np.ascontiguousarray(np.concatenate(idx_tiles, axis=1))  # [128, G*32]
        rough = np.ascontiguousarray(
            np.asarray(top_rough_scores_batch[rows], dtype=np.float32).reshape(1, -1)
            + np.float32(np.asarray(bout).reshape(-1)[0]))
        in_maps.append({
            "amen": amen, "ment": ment, "w1bt": w1bt, "w1abt": w1abt,
            "w1at": w1at, "w1pw": w1pw, "b1t": b1t, "woutt": woutt,
            "pwt": pwt, "idx": idx, "rough": rough,
        })
    return in_maps, B, n_tab


_NC_CACHE = {}


def kernel_with_results(all_mentions, mentions_batch, pw_batch, top_indices_batch,
                        top_rough_scores_batch, W1, b1, Wout, bout, **run_kwargs):
    in_maps, B, n_tab = prep_inputs(
        all_mentions, mentions_batch, pw_batch, top_indices_batch,
        top_rough_scores_batch, W1, b1, Wout, bout)
    key = (B, n_tab)
    if key not in _NC_CACHE:
        _NC_CACHE[key] = build_nc(B, n_tab)
    nc = _NC_CACHE[key]
    res = run_bass_kernel_spmd(nc, in_maps, list(range(N_CORES)), **run_kwargs)
    scores = np.concatenate([np.asarray(r["out"]) for r in res.results], axis=0)
    batch = scores.shape[0]
    full = np.empty((batch, N_ANTS + 1), np.float32)
    full[:, 0] = EPS
    full[:, 1:] = scores
    return full, res


def kernel(**inputs) -> np.ndarray:
    out, _ = kernel_with_results(**inputs)
    return out


# revision 12
# speedup vs baseline: 2.1821x; 1.0206x over previous
"""Trainium2 Bass kernel for the AnaphoricityScorer (coref pairwise FFNN scorer).

Math (per batch row i, antecedent slot t):
    b  = all_mentions[top_indices[i, t]]                    # gathered mention
    pair = [a_i, b, a_i * b, pw[i, t]]                      # 3*1024 + 64 features
    h  = leaky_relu(pair @ W1.T + b1, 0.01)                 # 1024 hidden
    ffnn = h @ Wout.T + bout                                # scalar
    score = rough[i, t] + ffnn
    out = concat([eps_col, scores], axis=1)                 # [batch, 65]

Distribution: pure data parallel over the batch dim across 8 NeuronCores
(no collectives). all_mentions and FFNN weights are replicated.

Per-core algorithm (B = 128 batch rows -> 8192 pair rows, groups of 512):
  - The a-term a_i @ W1a.T is identical for all 64 antecedents of row i, so it
    is computed once per batch row in a prologue (ha = mentions @ W1a.T + b1)
    and broadcast-added into the pair-row PSUM with a stride-0 DVE add.
  - b rows arrive transposed (features on partitions) straight from HBM via
    dma_gather(transpose=True), which is exactly the matmul rhs layout.
  - a*b is built by a DVE multiply against a stride-0 broadcast of mentions^T.
  - One PSUM accumulation of 17 matmuls per (row-group, hidden-tile):
    8 K-tiles of W1b, 8 of W1ab, 1 of W1pw (K=64).
  - Lrelu on ScalarE evicts PSUM -> SBUF bf16; the Wout reduction is a
    K=128, M=1 matmul accumulated over the 8 hidden tiles.
  - bout + rough scores are added on DVE; one DMA out per core.

Everything is bf16 on the TensorEngine with fp32 PSUM accumulation.
"""

import sys

for _p in ("/opt/trn_rl_repo",):
    if _p not in sys.path:
        sys.path.append(_p)

import numpy as np
import ml_dtypes

import concourse.bacc as bacc
import concourse.mybir as mybir
from concourse.tile import TileContext
from concourse.bass_utils import run_bass_kernel_spmd

BF16 = mybir.dt.bfloat16
F32 = mybir.dt.float32
I16 = mybir.dt.int16
FP8 = mybir.dt.float8e4

USE_FP8 = True       # b/ab blocks in fp8-e4m3 DoubleRow (2 k-tiles per matmul)
FP8_SCALE = 512.0    # weight pre-scale so 0.02-magnitude weights leave fp8 denormals

N_CORES = 8
EMB = 1024
HID = 1024
N_ANTS = 64
PW = 64
EPS = 1e-7
GRP = 512          # pair rows per group (= 8 batch rows)
ROWS_PER_GRP = 8   # batch rows per group


def build_nc(B: int, n_tab: int):
    """Build the per-core Bass graph. B = batch rows per core."""
    G = (B * N_ANTS) // GRP  # number of row groups
    FC = EMB // 128          # 8 feature k-tiles per 1024-feature block
    NT = HID // 128          # 8 hidden tiles

    nc = bacc.Bacc("TRN2")
    amen = nc.declare_dram_parameter("amen", [n_tab, EMB], BF16, isOutput=False)
    ment = nc.declare_dram_parameter("ment", [128, FC, B], BF16, isOutput=False)
    wdt = FP8 if USE_FP8 else BF16
    w1bt = nc.declare_dram_parameter("w1bt", [128, FC, HID], wdt, isOutput=False)
    w1abt = nc.declare_dram_parameter("w1abt", [128, FC, HID], wdt, isOutput=False)
    w1at = nc.declare_dram_parameter("w1at", [128, FC, HID], BF16, isOutput=False)
    w1pw = nc.declare_dram_parameter("w1pw", [128, HID], BF16, isOutput=False)
    woutt = nc.declare_dram_parameter("woutt", [128, NT], BF16, isOutput=False)
    pwt = nc.declare_dram_parameter("pwt", [128, B * N_ANTS], BF16, isOutput=False)
    idx = nc.declare_dram_parameter("idx", [128, G * (GRP // 16)], I16, isOutput=False)
    rough = nc.declare_dram_parameter("rough", [1, B * N_ANTS], F32, isOutput=False)
    out = nc.declare_dram_parameter("out", [B, N_ANTS], F32, isOutput=True)

    with TileContext(nc) as tc:
        with (
            tc.tile_pool(name="const", bufs=1) as const,
            tc.tile_pool(name="btp", bufs=5) as btp,
            tc.tile_pool(name="abtp", bufs=4) as abtp,
            tc.tile_pool(name="bt8p", bufs=4) as bt8p,
            tc.tile_pool(name="wgp", bufs=3) as wgp,
            tc.tile_pool(name="htp", bufs=10) as htp,
            tc.tile_pool(name="rpool", bufs=3) as rpool,
            tc.tile_pool(name="spool", bufs=2) as spool,
            tc.tile_pool(name="psum", bufs=3, space="PSUM") as psum_pool,
            tc.tile_pool(name="psum_s", bufs=2, space="PSUM") as psum_s_pool,
        ):
            # ---- resident loads (gather + prologue deps first) ------------
            idx_t = const.tile([128, G * (GRP // 16)], I16)
            nc.sync.dma_start(idx_t[:], idx[:, :])
            ment_t = const.tile([128, FC, B], BF16)
            nc.sync.dma_start(ment_t[:], ment[:, :, :])
            w1at_t = const.tile([128, FC, HID], BF16)
            nc.sync.dma_start(w1at_t[:], w1at[:, :, :])
            w1bt_t = const.tile([128, FC, HID], wdt)
            nc.sync.dma_start(w1bt_t[:], w1bt[:, :, :])
            w1abt_t = const.tile([128, FC, HID], wdt)
            nc.sync.dma_start(w1abt_t[:], w1abt[:, :, :])
            w1pw_t = const.tile([128, HID], BF16)
            nc.sync.dma_start(w1pw_t[:], w1pw[:, :])
            woutt_t = const.tile([128, NT], BF16)
            nc.sync.dma_start(woutt_t[:], woutt[:, :])
            pwt_t = const.tile([128, B * N_ANTS], BF16)
            nc.sync.dma_start(pwt_t[:], pwt[:, :])
            # ---- prologue: ha = mentions @ (W1a*S).T, rows-on-partitions --
            # ha2r regroups ha so group g's 8 batch rows sit on partitions
            # 64..71 of the per-group weight tile wg (spliced below); the
            # static pwt operand carries one-hot rows that select the batch
            # row, folding the a-term (and b1 via an all-ones row) into the
            # pw matmul for free.
            ha2 = const.tile([B, HID], BF16)
            for half in range(HID // 512):
                pp = psum_pool.tile([B, 512], F32)
                for fc in range(FC):
                    nc.tensor.matmul(
                        pp[:],
                        ment_t[:, fc, :],
                        w1at_t[:, fc, half * 512:(half + 1) * 512],
                        start=(fc == 0),
                        stop=(fc == FC - 1),
                    )
                nc.scalar.activation(
                    ha2[:, half * 512:(half + 1) * 512], pp[:],
                    mybir.ActivationFunctionType.Identity,
                )
            ha2_dram = nc.dram_tensor("ha2_scratch", [B, HID], BF16)
            nc.sync.dma_start(ha2_dram[:, :], ha2[:])
            ha2r = const.tile([8, G, HID], BF16)
            nc.sync.dma_start(
                ha2r[:],
                ha2_dram[:, :].rearrange("(g q) n -> q g n", q=ROWS_PER_GRP),
            )

            # ---- main loop over row groups --------------------------------
            # Software-pipelined emission: the gather + a*b multiplies for
            # group g+1 are emitted BEFORE group g's matmuls so the DVE
            # stream reaches them early, and each (g, nt) second-matmul is
            # deferred by one nt so its ht dependency never stalls PE.
            def produce_group(g):
                r0 = g * ROWS_PER_GRP
                rtile = rpool.tile([1, GRP], F32)
                nc.sync.dma_start(rtile[:], rough[0:1, g * GRP:(g + 1) * GRP])
                bt = btp.tile([128, FC, GRP], BF16)
                nc.gpsimd.dma_gather(
                    bt[:], amen[:, :],
                    idx_t[:, g * (GRP // 16):(g + 1) * (GRP // 16)],
                    GRP, GRP, EMB, transpose=True,
                )
                abt = abtp.tile([128, FC, GRP], FP8 if USE_FP8 else BF16)
                a_b = ment_t[:, :, r0:r0 + ROWS_PER_GRP]
                for fc in range(FC):
                    nc.vector.tensor_mul(
                        abt[:, fc, :].rearrange("p (a b) -> p a b", a=ROWS_PER_GRP),
                        bt[:, fc, :].rearrange("p (a b) -> p a b", a=ROWS_PER_GRP),
                        a_b[:, fc, :].unsqueeze(2).to_broadcast(
                            [128, ROWS_PER_GRP, N_ANTS]),
                    )
                if USE_FP8:
                    bt8 = bt8p.tile([128, FC, GRP], FP8)
                    for fc in range(FC):
                        nc.scalar.activation(
                            bt8[:, fc, :], bt[:, fc, :],
                            mybir.ActivationFunctionType.Identity)
                    bt = bt8
                wg = wgp.tile([128, HID], BF16)
                nc.vector.tensor_copy(wg[:], w1pw_t[:])
                nc.vector.tensor_copy(wg[64:72, :], ha2r[:, g, :])
                return bt, abt, rtile, wg

            def emit_batch(ps4, hts, nts, start):
                # 4 M=1 matmuls packed into distinct PE column groups -- they
                # execute concurrently in the array (one per 32-col strip)
                for nt_i, ht_i in zip(nts, hts):
                    j = nt_i % 4
                    nc.tensor.matmul(
                        ps4[32 * j:32 * j + 1, :], woutt_t[:, nt_i:nt_i + 1],
                        ht_i[:], tile_position=(0, 32 * j),
                        start=start, stop=not start,
                    )

            def finalize_group(ps4, p_g, p_rtile):
                # DVE may read at most one PSUM operand per op: chain the four
                # column-group partial rows through SBUF
                t1 = spool.tile([1, GRP], F32)
                nc.vector.tensor_add(t1[:], ps4[0:1, :], p_rtile[:])
                t2 = spool.tile([1, GRP], F32)
                nc.vector.tensor_add(t2[:], ps4[32:33, :], t1[:])
                t3 = spool.tile([1, GRP], F32)
                nc.vector.tensor_add(t3[:], ps4[64:65, :], t2[:])
                stile = spool.tile([1, GRP], F32)
                nc.vector.tensor_add(stile[:], ps4[96:97, :], t3[:])
                nc.sync.dma_start(
                    out[p_g * ROWS_PER_GRP:(p_g + 1) * ROWS_PER_GRP, :].unsqueeze(0),
                    stile[:].rearrange("p (r c) -> p r c", r=ROWS_PER_GRP),
                )

            tiles = {0: produce_group(0)}
            rtiles = {}
            prev_group = None  # (g, ps4, hts) awaiting its second batch
            for g in range(G):
                r0 = g * ROWS_PER_GRP
                bt, abt, rtiles[g], wg = tiles.pop(g)
                if g + 1 < G:
                    tiles[g + 1] = produce_group(g + 1)
                hts = []
                ps4 = None
                for nt in range(NT):
                    ps = psum_pool.tile([128, GRP], F32)
                    nsl = slice(nt * 128, (nt + 1) * 128)
                    if USE_FP8:
                        for fc in range(0, FC, 2):
                            nc.tensor.matmul(
                                ps[:], w1bt_t[:, fc:fc + 2, nsl], bt[:, fc:fc + 2, :],
                                perf_mode=mybir.MatmulPerfMode.DoubleRow,
                                start=(fc == 0), stop=False,
                            )
                        for fc in range(0, FC, 2):
                            nc.tensor.matmul(
                                ps[:], w1abt_t[:, fc:fc + 2, nsl], abt[:, fc:fc + 2, :],
                                perf_mode=mybir.MatmulPerfMode.DoubleRow,
                                start=False, stop=False,
                            )
                    else:
                        for fc in range(FC):
                            nc.tensor.matmul(
                                ps[:], w1bt_t[:, fc, nsl], bt[:, fc, :],
                                start=(fc == 0), stop=False,
                            )
                        for fc in range(FC):
                            nc.tensor.matmul(
                                ps[:], w1abt_t[:, fc, nsl], abt[:, fc, :],
                                start=False, stop=False,
                            )
                    nc.tensor.matmul(
                        ps[:], wg[:, nsl],
                        pwt_t[:, g * GRP:(g + 1) * GRP],
                        start=False, stop=True,
                    )
                    ht = htp.tile([128, GRP], BF16)
                    nc.scalar.activation(
                        ht[:], ps[:],
                        mybir.ActivationFunctionType.Lrelu, alpha=0.01,
                        scale=(1.0 / FP8_SCALE) if USE_FP8 else 1.0,
                    )
                    hts.append(ht)
                    if nt == 1 and prev_group is not None:
                        p_g, p_ps4, p_hts = prev_group
                        emit_batch(p_ps4, p_hts[4:8], range(4, 8), start=False)
                        finalize_group(p_ps4, p_g, rtiles.pop(p_g))
                        prev_group = None
                    if nt == 5:
                        ps4 = psum_s_pool.tile([128, GRP], F32)
                        emit_batch(ps4, hts[0:4], range(0, 4), start=True)
                prev_group = (g, ps4, hts)
            # flush the last group's second batch
            p_g, p_ps4, p_hts = prev_group
            emit_batch(p_ps4, p_hts[4:8], range(4, 8), start=False)
            finalize_group(p_ps4, p_g, rtiles.pop(p_g))

    nc.compile()
    return nc


def prep_inputs(all_mentions, mentions_batch, pw_batch, top_indices_batch,
                top_rough_scores_batch, W1, b1, Wout, bout, n_cores=N_CORES):
    """Host-side marshalling: shard over batch, cast/transpose into the
    layouts the kernel expects. Returns (in_maps, B, n_tab, bout_val)."""
    bf = ml_dtypes.bfloat16
    batch = mentions_batch.shape[0]
    B = batch // n_cores
    n_tab = all_mentions.shape[0]
    FC = EMB // 128
    NT = HID // 128
    G = (B * N_ANTS) // GRP

    amen = np.ascontiguousarray(all_mentions.astype(bf))

    def wt_block(Wcols, scale=1.0, dtype=bf):
        # [1024, 1024] f32 block -> [128, FC, HID] (feature on partitions)
        wt = Wcols.T.reshape(FC, 128, HID).transpose(1, 0, 2) * scale
        if dtype is not bf:
            wt = np.clip(wt, -240.0, 240.0)
        return np.ascontiguousarray(wt.astype(dtype))

    S = FP8_SCALE if USE_FP8 else 1.0
    f8 = ml_dtypes.float8_e4m3
    wdt = f8 if USE_FP8 else bf
    w1at = wt_block(W1[:, 0:EMB], S)
    w1bt = wt_block(W1[:, EMB:2 * EMB], S, wdt)
    w1abt = wt_block(W1[:, 2 * EMB:3 * EMB], S, wdt)
    w1pw = np.zeros((128, HID), dtype=bf)
    w1pw[:PW] = (W1[:, 3 * EMB:3 * EMB + PW].T * S).astype(bf)
    w1pw[72] = (b1 * S).astype(bf)
    woutt = np.ascontiguousarray(Wout[0].reshape(NT, 128).T.astype(bf))

    in_maps = []
    for c in range(n_cores):
        rows = slice(c * B, (c + 1) * B)
        m_c = np.asarray(mentions_batch[rows], dtype=np.float32)       # [B, 1024]
        ment = np.ascontiguousarray(
            m_c.T.reshape(FC, 128, B).transpose(1, 0, 2).astype(bf))   # [128, FC, B]
        pw_c = np.asarray(pw_batch[rows], dtype=np.float32)            # [B, 64, 64]
        pwt = np.zeros((128, B * N_ANTS), dtype=bf)
        pwt[:PW] = pw_c.reshape(B * N_ANTS, PW).T.astype(bf)
        cols = np.arange(B * N_ANTS)
        for q in range(ROWS_PER_GRP):
            pwt[PW + q] = ((cols % GRP) // N_ANTS == q).astype(bf)
        pwt[72] = np.ones(B * N_ANTS, dtype=bf)
        idx_c = np.asarray(top_indices_batch[rows]).astype(np.int64).reshape(-1)
        idx_tiles = []
        for g in range(G):
            v = idx_c[g * GRP:(g + 1) * GRP].astype(np.int16)
            idx_tiles.append(np.tile(v.reshape(GRP // 16, 16).T, (8, 1)))
        idx = np.ascontiguousarray(np.concatenate(idx_tiles, axis=1))  # [128, G*32]
        rough = np.ascontiguousarray(
            np.asarray(top_rough_scores_batch[rows], dtype=np.float32).reshape(1, -1)
            + np.float32(np.asarray(bout).reshape(-1)[0]))
        in_maps.append({
            "amen": amen, "ment": ment, "w1bt": w1bt, "w1abt": w1abt,
            "w1at": w1at, "w1pw": w1pw, "woutt": woutt,
            "pwt": pwt, "idx": idx, "rough": rough,
        })
    return in_maps, B, n_tab


_NC_CACHE = {}


def kernel_with_results(all_mentions, mentions_batch, pw_batch, top_indices_batch,
                        top_rough_scores_batch, W1, b1, Wout, bout, **run_kwargs):
    in_maps, B, n_tab = prep_inputs(
        all_mentions, mentions_batch, pw_batch, top_indices_batch,
        top_rough_scores_batch, W1, b1, Wout, bout)
    key = (B, n_tab)
    if key not in _NC_CACHE:
        _NC_CACHE[key] = build_nc(B, n_tab)
    nc = _NC_CACHE[key]
    res = run_bass_kernel_spmd(nc, in_maps, list(range(N_CORES)), **run_kwargs)
    scores = np.concatenate([np.asarray(r["out"]) for r in res.results], axis=0)
    batch = scores.shape[0]
    full = np.empty((batch, N_ANTS + 1), np.float32)
    full[:, 0] = EPS
    full[:, 1:] = scores
    return full, res


def kernel(**inputs) -> np.ndarray:
    out, _ = kernel_with_results(**inputs)
    return out
